# revision 50
# baseline (speedup 1.0000x reference)
"""Trainium2 Bass kernel for nn_Attention (B=4, S=1024, DIM=1024, H=16, Dh=64).

Sharding: 8 cores = 4 batches x 2 head-groups (8 heads / 512 inner channels
each).  Each core computes q/k/v projections for its head shard, RoPE,
attention, and a partial output projection (its rows of Wo); the host sums
the two head-group partials per batch (the tensor-parallel all-reduce done
on host), adds the biases the device never sees (bo + bv@Wo -- softmax
weights sum to 1, so the v-bias shifts attn-out by exactly bv), and
re-applies the row mask.

Device dataflow (per core), matmul operands in fp16 (fp32 PSUM accumulate):
  input DMAs chunked in first-consumer order so the prologue's first
  matmuls start ~13us in while the weight tails stream ->
  Q^T,K^T = W^T @ x^T      (bias added on the PSUM->SBUF pass)
  RoPE on the first 64 flat channels only (reference rotates rot_dim=64 of
  the flat inner dim): qr = (q+b)*cos + P_rot@((q+b)*sin), P_rot on PE.
  Attention runs as 8 sequential (row-tile, q-half) passes of 8 key-chunk
  iterations each, with later row-tiles' projection matmuls zippered in:
  scores^T[k,q] = K_h @ Q_h^T   (K=64; the two heads of a pass issue on
                                 row groups 0/64 -> concurrent; the next
                                 pass's first tile is emitted at kt==7 so
                                 ACT never drains)
  P^T = exp(scores^T/8 + maskbias[k])  (ACT, one op per pass-iteration;
                                        key mask folded into the exp bias)
  attn^T[c,q] = V_aug^T @ P^T   (V_aug carries an all-ones 65th channel, so
                                 the matmul emits the softmax rowsum at PSUM
                                 partition 64 for free -- no separate
                                 rowsum matmuls)
  epilogue per pass: rowsum rows staged to SBUF (the custom-DVE reciprocal
  mangles offset single-partition PSUM reads) and attn spilled first (they
  gate the next pass's PSUM reuse), then reciprocal -> f16; normalization
  multiplies ride later passes via a DRAM-bounce partition-broadcast on
  the GpSimd queue; only the last q-half normalizes via PE broadcast.
  Phase 3: out[q,:] = attn^T.T @ Wo_shard as bare 4-matmul chains (biases
  on host), masked rows zeroed on the ACT PSUM->SBUF copy, f16 output DMA
  spread over three queues; qt 0-3 are emitted before the final-normalize
  broadcast so they overlap its DVE mults.
"""

import numpy as np

B, S, DIM, HEADS, HEAD_DIM = 4, 1024, 1024, 16, 64
INNER = HEADS * HEAD_DIM
HG = 2                      # head groups (tensor-parallel shards)
DSH = INNER // HG           # 512 inner channels per core
HSH = HEADS // HG           # 8 heads per core
NCORES = B * HG
KT = DIM // 128             # 8 contraction tiles
MT = DSH // 128             # 4 output row tiles for Q^T/K^T
ST = S // 128               # 8 seq tiles
MASK_NEG = -80.0

_CACHE = {}


def _build():
    import concourse.tile as tile
    from concourse import bacc, mybir

    f32 = mybir.dt.float32
    f16 = mybir.dt.float16
    f8 = mybir.dt.float8e4
    AF = mybir.ActivationFunctionType
    OP = mybir.AluOpType
    DR = mybir.MatmulPerfMode.DoubleRow

    nc = bacc.Bacc("TRN2", target_bir_lowering=False, debug=False)

    xT_d = nc.dram_tensor("xT", [128, KT, S], f16, kind="ExternalInput")
    wq_d = nc.dram_tensor("wq", [128, MT, KT, 128], f16, kind="ExternalInput")
    wk_d = nc.dram_tensor("wk", [128, MT, KT, 128], f16, kind="ExternalInput")
    wv_d = nc.dram_tensor("wv", [128, KT, DSH], f16, kind="ExternalInput")
    wo_d = nc.dram_tensor("wo", [128, MT, DIM], f16, kind="ExternalInput")
    bq_d = nc.dram_tensor("bq", [128, MT], f32, kind="ExternalInput")
    bk_d = nc.dram_tensor("bk", [128, MT], f32, kind="ExternalInput")
    cos_d = nc.dram_tensor("cos2", [128, S], f32, kind="ExternalInput")
    sin_d = nc.dram_tensor("sin2", [128, S], f32, kind="ExternalInput")
    prt_d = nc.dram_tensor("prt", [128, 128], f16, kind="ExternalInput")
    maskb_d = nc.dram_tensor("maskb", [128, ST], f32, kind="ExternalInput")
    mask01_d = nc.dram_tensor("mask01", [128, ST], f32, kind="ExternalInput")
    out_d = nc.dram_tensor("out", [S, DIM], f16, kind="ExternalOutput")

    with tile.TileContext(nc) as tc, \
         tc.tile_pool(name="persist", bufs=1) as persist:
        with tc.tile_pool(name="w1", bufs=1) as w1:
            # phase-1-only constants
            xT = w1.tile([128, KT, S], f16)
            wq = w1.tile([128, MT, KT, 128], f16)
            wk = w1.tile([128, MT, KT, 128], f16)
            wv = w1.tile([128, KT, DSH], f16)
            bq = w1.tile([128, MT], f32)
            bk = w1.tile([128, MT], f32)
            cos2 = w1.tile([128, S], f32)
            sin2 = w1.tile([128, S], f32)
            prt = w1.tile([128, 128], f16)
            wo = persist.tile([128, MT, DIM], f16)
            maskb = persist.tile([128, ST], f32)
            mask01 = persist.tile([128, ST], f32)
            # DMA order = first-consumer order: the prologue's K0/Q0 chains
            # need wk/wq row-tile 0 and the low kt slices of xT first, so
            # those are issued first (on separate engine queues so chunks
            # stream concurrently); bulk weight tails follow.
            nc.scalar.dma_start(out=wk[:, 0], in_=wk_d.ap()[:, 0])
            nc.gpsimd.dma_start(out=wq[:, 0], in_=wq_d.ap()[:, 0])
            for j in range(4):
                nc.sync.dma_start(out=xT[:, 2 * j:2 * j + 2],
                                  in_=xT_d.ap()[:, 2 * j:2 * j + 2])
            for t, d in [(bq, bq_d), (bk, bk_d),
                         (cos2, cos_d), (sin2, sin_d), (prt, prt_d),
                         (maskb, maskb_d), (mask01, mask01_d)]:
                nc.gpsimd.dma_start(out=t[:], in_=d.ap())
            nc.scalar.dma_start(out=wv[:], in_=wv_d.ap())
            ones_f = w1.tile([128, S], f32)
            nc.vector.memset(ones_f[:], 1.0)

            qT = persist.tile([128, MT, S], f16)
            kT = persist.tile([128, MT, S], f16)
            # V with an all-ones 65th channel: the PV matmul emits the
            # softmax rowsum at PSUM partition 64 for free (M=65);
            # whole-tile memset (strided column memsets misfire), V spills
            # overwrite channels 0-63, leaving channel 64 == 1.0
            vv = persist.tile([128, ST, HSH, 72], f16)
            nc.vector.memset(vv[:], 1.0)
            ones4 = persist.tile([97, HEAD_DIM], f16)
            nc.vector.tensor_copy(ones4[:], ones_f[0:97, 0:HEAD_DIM])

            # ---- phases 1+2: projections zippered into attention ----
            # K0/Q0/V run as a prologue; each attention (mt, c2) pass's
            # inner loop then carries later projection matmuls so PE fills
            # the gaps while ACT streams exps.
            attU = persist.tile([128, MT, S], f16)
            # rowsum reciprocals at partitions 32*(hh*2+c2)
            recq = persist.tile([97, MT, 512], f16)
            rs_sb = persist.tile([97, MT, 512], f32)
            recf = persist.tile([97, MT, 512], f32)
            recd = nc.dram_tensor("recd", [97, MT, 512], f16)

            with tc.tile_pool(name="p1ps", bufs=2, space="PSUM") as p1ps, \
                 tc.tile_pool(name="p1sb", bufs=3) as p1sb, \
                 tc.tile_pool(name="p2r", bufs=2) as p2r:

                def rope_apply(dst, b, c2, ps, pppool):
                    # row-tile 0 only: RoPE on the first 64 flat channels
                    # (rows 64-127 and the hg=1 core get identity via
                    # cos=1/sin=0 from the host).
                    sl = slice(c2 * 512, (c2 + 1) * 512)
                    sinp = p1sb.tile([128, 512], f16, tag="sinp", name="sinp")
                    nc.vector.scalar_tensor_tensor(
                        sinp[:], ps[:], b[:, 0:1],
                        sin2[:, sl], op0=OP.add, op1=OP.mult)
                    cosp = p1sb.tile([128, 512], f32, tag="cosp", name="cosp")
                    nc.vector.scalar_tensor_tensor(
                        cosp[:], ps[:], b[:, 0:1],
                        cos2[:, sl], op0=OP.add, op1=OP.mult)
                    pp = pppool.tile([128, 512], f32, tag="pp", name="pp")
                    nc.tensor.matmul(out=pp[:], lhsT=prt[:], rhs=sinp[:],
                                     start=True, stop=True)
                    nc.vector.tensor_tensor(
                        dst[:, 0, sl], cosp[:], pp[:], op=OP.add)

                def proj_v(st, pool):
                    # bv is folded in on the host: softmax weights sum to 1,
                    # so attn-out = attU/denom + bv exactly; bv@Wo joins bo.
                    ps = pool.tile([128, DSH], f32, tag="vps", name="ps")
                    for kt in range(KT):
                        nc.tensor.matmul(
                            out=ps[:],
                            lhsT=xT[:, kt, st * 128:(st + 1) * 128],
                            rhs=wv[:, kt, :],
                            start=(kt == 0), stop=(kt == KT - 1))
                    # spill on DVE: ACT's FIFO must stay clear so the first
                    # attention exps aren't queued behind these copies
                    nc.vector.tensor_copy(
                        vv[:, st, :, 0:HEAD_DIM],
                        ps[:].rearrange("p (h d) -> p h d", h=HSH))

                def proj_gen(mt):
                    # generator: one projection matmul per next(); bias-add
                    # epilogue rides with each group's final matmul.
                    for dst, w, b in ((kT, wk, bk), (qT, wq, bq)):
                        for c2 in range(2):
                            sl = slice(c2 * 512, (c2 + 1) * 512)
                            ps = p1ps.tile([128, 512], f32, tag="ps",
                                           name="ps")
                            for kt in range(KT):
                                nc.tensor.matmul(
                                    out=ps[:], lhsT=w[:, mt, kt, :],
                                    rhs=xT[:, kt, sl],
                                    start=(kt == 0), stop=(kt == KT - 1))
                                if kt == KT - 1:
                                    nc.vector.tensor_scalar(
                                        dst[:, mt, sl], ps[:], b[:, mt:mt + 1],
                                        None, op0=OP.add)
                                yield
                    while True:
                        yield

                # prologue: row-tile 0 projections + all of V, with a
                # deep PSUM pool (banks are free until the attention pools
                # open); all 32 K0/Q0 matmuls run dense, then the RoPE
                # chains consume the held PSUM tiles while V streams.
                with tc.tile_pool(name="p1pp", bufs=1,
                                  space="PSUM") as p1pp, \
                     tc.tile_pool(name="vps", bufs=5,
                                  space="PSUM") as vps:
                    kq_ps = []
                    for dst, w, b in ((kT, wk, bk), (qT, wq, bq)):
                        for c2 in range(2):
                            sl = slice(c2 * 512, (c2 + 1) * 512)
                            ps = vps.tile([128, 512], f32, tag="vps",
                                          name="ps")
                            for kt in range(KT):
                                nc.tensor.matmul(
                                    out=ps[:], lhsT=w[:, 0, kt, :],
                                    rhs=xT[:, kt, sl],
                                    start=(kt == 0), stop=(kt == KT - 1))
                            kq_ps.append((dst, b, c2, ps))
                    proj_v(0, vps)
                    for dst, b, c2, ps in kq_ps:
                        rope_apply(dst, b, c2, ps, p1pp)
                    for st in range(1, ST):
                        proj_v(st, vps)

                def normalize(mt, c2):
                    # DRAM bounce on the gpsimd DMA queue partition-
                    # broadcasts each head's 1/rowsum row; the scale runs on
                    # the otherwise-idle GpSimd engine to keep DVE clear.
                    qsl = slice(c2 * 512, (c2 + 1) * 512)
                    nc.gpsimd.dma_start(
                        out=recd.ap()[32 * c2:32 * c2 + 65:64, mt, :],
                        in_=recq[32 * c2:32 * c2 + 65:64, mt, :])
                    for hh in range(2):
                        ph = hh * 64
                        r = 32 * (hh * 2 + c2)
                        rb = p2r.tile([128, 512], f32, tag="rb", name="rb")
                        nc.gpsimd.dma_start(
                            out=rb[ph:ph + 64],
                            in_=recd.ap()[r:r + 1,
                                          mt, :].partition_broadcast(HEAD_DIM))
                        nc.gpsimd.tensor_tensor(
                            attU[ph:ph + 64, mt, qsl],
                            attU[ph:ph + 64, mt, qsl],
                            rb[ph:ph + 64], op=OP.mult)

                with tc.tile_pool(name="p2sc", bufs=2, space="PSUM") as p2sc, \
                     tc.tile_pool(name="p2at", bufs=1, space="PSUM") as p2at, \
                     tc.tile_pool(name="p2sb", bufs=2) as p2sb:

                    def emit_scores(mt, kt, c2):
                        # one [128,1024] tile per (q-half, key-chunk):
                        # h0 -> bank-half 0:512, h1 -> 512:1024 (adjacent
                        # row-group pair -> concurrent); both heads share
                        # the per-key exp bias so ONE exp covers the tile.
                        qsl = slice(c2 * 512, (c2 + 1) * 512)
                        sch = p2sc.tile([128, S], f32, tag="scc", name="scc")
                        for hh in range(2):
                            ph = hh * 64
                            nc.tensor.matmul(
                                out=sch[:, hh * 512:hh * 512 + 512],
                                lhsT=kT[ph:ph + 64, mt,
                                        kt * 128:(kt + 1) * 128],
                                rhs=qT[ph:ph + 64, mt, qsl],
                                start=True, stop=True,
                                tile_position=(ph, 0))
                        return sch

                    # q-halves run sequentially (16 key-chunk iterations per
                    # row-tile): DoubleRow PV must write PSUM partition 0, so
                    # only one q-half's pair of [65,512] accumulators fits
                    # alongside the double-buffered score tiles.
                    # nonzero through the last iterations so the PE has
                    # filler while it waits on the final exps of a pass
                    nproj_it = (3, 3, 3, 3, 2, 2, 2, 2, 2, 2, 2, 2, 1, 1, 1, 1)
                    passes = [(mt, c2) for mt in range(MT) for c2 in range(2)]
                    # the wk/wq row-tile tails (first consumed by the
                    # zippered projections at pass 0) and phase-3-only wo
                    # are issued here so they do not compete with x/wv
                    # during the bandwidth-bound prologue window
                    nc.gpsimd.dma_start(out=wk[:, 1:4], in_=wk_d.ap()[:, 1:4])
                    nc.gpsimd.dma_start(out=wq[:, 1:4], in_=wq_d.ap()[:, 1:4])
                    nc.scalar.dma_start(out=wo[:], in_=wo_d.ap())
                    pending_recip = []
                    sch_next = emit_scores(0, 0, 0)
                    for pi, (mt, c2) in enumerate(passes):
                        if c2 == 0:
                            gen = proj_gen(mt + 1) if mt + 1 < MT else iter(
                                lambda: None, 0)  # infinite no-op iterator
                        if True:
                            qsl = slice(c2 * 512, (c2 + 1) * 512)
                            at = {hh: p2at.tile([65, 512], f32,
                                                name=f"at{hh}", tag=f"at{hh}")
                                  for hh in range(2)}
                            sch = sch_next
                            pt2 = None
                            for kt in range(ST):
                                pt2 = p2sb.tile([128, S], f16,
                                                tag="ptc", name="ptc")
                                nc.scalar.activation(
                                    pt2[:], sch[:], AF.Exp,
                                    bias=maskb[:, kt:kt + 1], scale=0.125)
                                first, last = (kt == 0), (kt == ST - 1)
                                if kt == 1 and pending_recip:
                                    # previous pass's reciprocal chain runs
                                    # here so it never gates the DVE at the
                                    # pass boundary; its consumer (the kt==2
                                    # DMA-bounce normalize) is still ahead
                                    pr = pending_recip.pop(0)
                                    nc.vector.reciprocal_approx_fast(
                                        recf[:, pr, :], rs_sb[:, pr, :])
                                    nc.vector.tensor_copy(recq[:, pr, :],
                                                          recf[:, pr, :])
                                if not last:
                                    sch = emit_scores(mt, kt + 1, c2)
                                elif pi + 1 < len(passes):
                                    # cross-pass prefetch: the next pass's
                                    # first score tile, so ACT rolls into
                                    # its exps with no pipeline drain
                                    sch_next = emit_scores(
                                        passes[pi + 1][0], 0,
                                        passes[pi + 1][1])
                                for _ in range(nproj_it[c2 * ST + kt]):
                                    next(gen)
                                for hh in range(2):  # PV+rowsum (M=65)
                                    nc.tensor.matmul(
                                        out=at[hh][:],
                                        lhsT=vv[:, kt, mt * 2 + hh, 0:65],
                                        rhs=pt2[:, hh * 512:hh * 512 + 512],
                                        start=first, stop=last)
                                if kt == 2 and c2 == 0 and mt > 0:
                                    normalize(mt - 1, 1)
                                if kt == 2 and c2 == 1:
                                    normalize(mt, 0)
                            # epilogue for this (mt, c2): stage the PSUM
                            # rowsum rows to SBUF (the custom-DVE recip
                            # mangles offset single-partition PSUM reads),
                            # one full-height reciprocal (junk partitions
                            # between the used rows are never read), then
                            # spill the attn columns
                            # rowsum staging + attn spills first -- they are
                            # the last readers of the PSUM accumulators, so
                            # the next pass's PV chain unblocks as soon as
                            # they retire; the reciprocal chain (only needed
                            # at the next pass's kt==2 bounce) runs after.
                            last_half = (mt == MT - 1 and c2 == 1)
                            for hh in range(2):
                                r = 32 * (hh * 2 + c2)
                                ph = hh * 64
                                nc.vector.tensor_copy(
                                    rs_sb[r:r + 1, mt, :], at[hh][64:65, :])
                                # one spill rides ACT when the следующий pass
                                # still has exp slack (mt0-2), halving the
                                # DVE work that gates PSUM reuse at the
                                # pass boundary
                                on_act = ((last_half and hh == 0) or
                                          (not last_half and hh == 1 and
                                           pi < 5))
                                if on_act:
                                    nc.scalar.activation(
                                        attU[ph:ph + 64, mt, qsl],
                                        at[hh][0:64, :], AF.Copy)
                                else:
                                    nc.vector.tensor_copy(
                                        attU[ph:ph + 64, mt, qsl],
                                        at[hh][0:64, :])
                            if last_half:
                                nc.vector.reciprocal_approx_fast(
                                    recf[:, mt, :], rs_sb[:, mt, :])
                                nc.vector.tensor_copy(recq[:, mt, :],
                                                      recf[:, mt, :])
                            else:
                                pending_recip.append(mt)

        # ---- phase 3: output projection -------------------------------
        # bo (and bv@Wo) are added on the host, so each (qt, c2) tile is a
        # bare 4-matmul accumulation chain; f16 output halves the out DMA.
        with tc.tile_pool(name="p3ps", bufs=8, space="PSUM") as p3ps, \
             tc.tile_pool(name="p3sb", bufs=3) as p3sb:
            # qt 0-3 read only attU columns 0-511 (normalized during the
            # c2=1 pass via the DMA bounce), so their chains are emitted
            # FIRST; the c2=1-half broadcast normalize runs on DVE under
            # them, unblocking qt 4-7.
            def final_norm():
                for hh in range(2):
                    ph = hh * 64
                    r = 32 * (hh * 2 + 1)
                    rbps = p3ps.tile([HEAD_DIM, 512], f32, tag="ps3",
                                     name="rbps")
                    nc.tensor.matmul(
                        out=rbps[:], lhsT=ones4[r:r + 1, :],
                        rhs=recq[r:r + 1, MT - 1, :],
                        start=True, stop=True, tile_position=(r, 0))
                    nc.vector.tensor_tensor(
                        attU[ph:ph + 64, MT - 1, 512:1024],
                        attU[ph:ph + 64, MT - 1, 512:1024],
                        rbps[:], op=OP.mult)
            dmaq = [nc.sync, nc.scalar, nc.gpsimd]
            for qt in list(range(4)) + ["norm"] + list(range(4, ST)):
                if qt == "norm":
                    final_norm()
                    continue
                ob = p3sb.tile([128, DIM], f16, tag="ob")
                for c2 in range(DIM // 512):
                    nsl = slice(c2 * 512, (c2 + 1) * 512)
                    ps = p3ps.tile([128, 512], f32, tag="ps3", name="ps3")
                    for mt in range(MT):
                        nc.tensor.matmul(
                            out=ps[:],
                            lhsT=attU[:, mt, qt * 128:(qt + 1) * 128],
                            rhs=wo[:, mt, nsl],
                            start=(mt == 0), stop=(mt == MT - 1))
                    # masked-row zeroing fused into the PSUM->SBUF move, on
                    # ACT (idle in this phase) to keep DVE off the path
                    nc.scalar.activation(
                        ob[:, nsl], ps[:], AF.Copy,
                        scale=mask01[:, qt:qt + 1])
                    dmaq[(qt * 2 + c2) % 3].dma_start(
                        out=out_d.ap()[qt * 128:(qt + 1) * 128, nsl],
                        in_=ob[:, nsl])

    nc.compile()
    return nc


def _get_nc():
    if "nc" not in _CACHE:
        _CACHE["nc"] = _build()
    return _CACHE["nc"]


def _prep_inputs(x, mask, freqs, Wq, bq, Wk, bk, Wv, bv, Wo, bo):
    f = np.asarray(freqs, np.float32)[0]              # [S, HEAD_DIM]
    # reference rotates only the first rot_dim=64 channels of the FLAT
    # inner dim -> rows 0-63 of row-tile 0 on the hg=0 core; everything
    # else is identity (cos=1, sin=0).
    cos2 = np.ones((128, S), np.float32)
    sin2 = np.zeros((128, S), np.float32)
    cos2[0:HEAD_DIM] = np.cos(f.T)
    sin2[0:HEAD_DIM] = np.sin(f.T)
    ident = np.ones((128, S), np.float32)
    identz = np.zeros((128, S), np.float32)

    prt = np.zeros((128, 128), np.float16)            # P_rot^T
    i = np.arange(0, 128, 2)
    prt[i + 1, i] = -1.0                              # P_rot[2i, 2i+1] = -1
    prt[i, i + 1] = 1.0                               # P_rot[2i+1, 2i] = +1

    def lhsT_w(w):                                    # [DIM, DSH] -> lhsT tiles
        # mt-major so the row-tile-0 slice is one contiguous DMA chunk
        return np.ascontiguousarray(
            w.reshape(KT, 128, MT, 128).transpose(1, 2, 0, 3)).astype(np.float16)

    def col(b):                                       # [DSH] -> [128, MT]
        return np.ascontiguousarray(b.reshape(MT, 128).T.astype(np.float32))

    in_maps = []
    for b in range(B):
        xT = np.ascontiguousarray(
            np.asarray(x[b], np.float32).T.reshape(KT, 128, S)
            .transpose(1, 0, 2)).astype(np.float16)
        m = np.asarray(mask[b])
        maskb = np.ascontiguousarray(
            np.where(m, 0.0, MASK_NEG).astype(np.float32).reshape(ST, 128).T)
        mask01 = np.ascontiguousarray(
            m.astype(np.float32).reshape(ST, 128).T)
        for hg in range(HG):
            dsl = slice(hg * DSH, (hg + 1) * DSH)
            in_maps.append({
                "xT": xT,
                "wq": lhsT_w(np.asarray(Wq, np.float32)[:, dsl]),
                "wk": lhsT_w(np.asarray(Wk, np.float32)[:, dsl]),
                "wv": np.ascontiguousarray(
                    np.asarray(Wv, np.float32)[:, dsl]
                    .reshape(KT, 128, DSH).transpose(1, 0, 2)).astype(np.float16),
                "wo": np.ascontiguousarray(
                    np.asarray(Wo, np.float32)[dsl, :]
                    .reshape(MT, 128, DIM).transpose(1, 0, 2)).astype(np.float16),
                "bq": col(np.asarray(bq, np.float32)[dsl]),
                "bk": col(np.asarray(bk, np.float32)[dsl]),
                "cos2": cos2 if hg == 0 else ident,
                "sin2": sin2 if hg == 0 else identz,
                "prt": prt,
                "maskb": maskb, "mask01": mask01,
            })
    return in_maps


def run(trace=False, **inputs):
    from concourse import bass_utils
    if trace:
        _install_ntff_hook()
    nc = _get_nc()
    in_maps = _prep_inputs(**inputs)
    res = bass_utils.run_bass_kernel_spmd(
        nc, in_maps, core_ids=list(range(NCORES)), trace=trace)
    # host epilogue: sum the two head-group partials and add the biases the
    # device never sees (bo, plus bv@Wo -- softmax weights sum to 1, so the
    # v-bias shifts every attn-out row by exactly bv); masked rows are 0.
    bias = (np.asarray(inputs["bo"], np.float32)
            + np.asarray(inputs["bv"], np.float32)
            @ np.asarray(inputs["Wo"], np.float32))
    mask = np.asarray(inputs["mask"])
    out = np.empty((B, S, DIM), np.float32)
    for b in range(B):
        s = (res.results[2 * b]["out"].astype(np.float32)
             + res.results[2 * b + 1]["out"].astype(np.float32) + bias)
        out[b] = np.where(mask[b][:, None], s, 0.0)
    return out, res


def kernel(**inputs):
    out, _ = run(trace=False, **inputs)
    return out


def _install_ntff_hook():
    """Register the axon NTFF profiling hook missing from the antenv stub."""
    import sys, types
    try:
        import antenv.axon_hooks  # noqa: F401
        return
    except ImportError:
        pass
    from trn_agent_boot.trn_boot import _ntff_profile_via_ctypes
    hook = _ntff_profile_via_ctypes('/opt/axon/libaxon_pjrt.so')
    mod = types.ModuleType('antenv.axon_hooks')
    mod.get_axon_ntff_profile_hook = lambda: hook
    mod.set_axon_ntff_profile_hook = lambda h: None
    sys.modules['antenv.axon_hooks'] = mod



# revision 51
# speedup vs baseline: 1.0180x; 1.0180x over previous
"""Trainium2 Bass kernel for nn_Attention (B=4, S=1024, DIM=1024, H=16, Dh=64).

Sharding: 8 cores = 4 batches x 2 head-groups (8 heads / 512 inner channels
each).  Each core computes q/k/v projections for its head shard, RoPE,
attention, and a partial output projection (its rows of Wo); the host sums
the two head-group partials per batch (the tensor-parallel all-reduce done
on host), adds the biases the device never sees (bo + bv@Wo -- softmax
weights sum to 1, so the v-bias shifts attn-out by exactly bv), and
re-applies the row mask.

Device dataflow (per core), matmul operands in fp16 (fp32 PSUM accumulate):
  input DMAs chunked in first-consumer order so the prologue's first
  matmuls start ~13us in while the weight tails stream ->
  Q^T,K^T = W^T @ x^T      (bias added on the PSUM->SBUF pass)
  RoPE on the first 64 flat channels only (reference rotates rot_dim=64 of
  the flat inner dim): qr = (q+b)*cos + P_rot@((q+b)*sin), P_rot on PE.
  Attention runs as 8 sequential (row-tile, q-half) passes of 8 key-chunk
  iterations each, with later row-tiles' projection matmuls zippered in:
  scores^T[k,q] = K_h @ Q_h^T   (K=64; the two heads of a pass issue on
                                 row groups 0/64 -> concurrent; the next
                                 pass's first tile is emitted at kt==7 so
                                 ACT never drains)
  P^T = exp(scores^T/8 + maskbias[k])  (ACT, one op per pass-iteration;
                                        key mask folded into the exp bias)
  attn^T[c,q] = V_aug^T @ P^T   (V_aug carries an all-ones 65th channel, so
                                 the matmul emits the softmax rowsum at PSUM
                                 partition 64 for free -- no separate
                                 rowsum matmuls)
  epilogue per pass: rowsum rows staged to SBUF (the custom-DVE reciprocal
  mangles offset single-partition PSUM reads) and attn spilled first (they
  gate the next pass's PSUM reuse), then reciprocal -> f16; normalization
  multiplies ride later passes via a DRAM-bounce partition-broadcast on
  the GpSimd queue; only the last q-half normalizes via PE broadcast.
  Phase 3: out[q,:] = attn^T.T @ Wo_shard as bare 4-matmul chains (biases
  on host), masked rows zeroed on the ACT PSUM->SBUF copy, f16 output DMA
  spread over three queues; qt 0-3 are emitted before the final-normalize
  broadcast so they overlap its DVE mults.
"""

import numpy as np

B, S, DIM, HEADS, HEAD_DIM = 4, 1024, 1024, 16, 64
INNER = HEADS * HEAD_DIM
HG = 2                      # head groups (tensor-parallel shards)
DSH = INNER // HG           # 512 inner channels per core
HSH = HEADS // HG           # 8 heads per core
NCORES = B * HG
KT = DIM // 128             # 8 contraction tiles
MT = DSH // 128             # 4 output row tiles for Q^T/K^T
ST = S // 128               # 8 seq tiles
MASK_NEG = -80.0

_CACHE = {}


def _build():
    import concourse.tile as tile
    from concourse import bacc, mybir

    f32 = mybir.dt.float32
    f16 = mybir.dt.float16
    f8 = mybir.dt.float8e4
    AF = mybir.ActivationFunctionType
    OP = mybir.AluOpType
    DR = mybir.MatmulPerfMode.DoubleRow

    nc = bacc.Bacc("TRN2", target_bir_lowering=False, debug=False)

    xT_d = nc.dram_tensor("xT", [128, KT, S], f16, kind="ExternalInput")
    wq_d = nc.dram_tensor("wq", [128, MT, KT, 128], f16, kind="ExternalInput")
    wk_d = nc.dram_tensor("wk", [128, MT, KT, 128], f16, kind="ExternalInput")
    wv_d = nc.dram_tensor("wv", [128, KT, DSH], f16, kind="ExternalInput")
    wo_d = nc.dram_tensor("wo", [128, MT, DIM], f16, kind="ExternalInput")
    bq_d = nc.dram_tensor("bq", [128, MT], f32, kind="ExternalInput")
    bk_d = nc.dram_tensor("bk", [128, MT], f32, kind="ExternalInput")
    cos_d = nc.dram_tensor("cos2", [128, S], f32, kind="ExternalInput")
    sin_d = nc.dram_tensor("sin2", [128, S], f32, kind="ExternalInput")
    prt_d = nc.dram_tensor("prt", [128, 128], f16, kind="ExternalInput")
    maskb_d = nc.dram_tensor("maskb", [128, ST], f32, kind="ExternalInput")
    mask01_d = nc.dram_tensor("mask01", [128, ST], f32, kind="ExternalInput")
    out_d = nc.dram_tensor("out", [S, DIM], f16, kind="ExternalOutput")

    with tile.TileContext(nc) as tc, \
         tc.tile_pool(name="persist", bufs=1) as persist:
        with tc.tile_pool(name="w1", bufs=1) as w1:
            # phase-1-only constants
            xT = w1.tile([128, KT, S], f16)
            wq = w1.tile([128, MT, KT, 128], f16)
            wk = w1.tile([128, MT, KT, 128], f16)
            wv = w1.tile([128, KT, DSH], f16)
            bq = w1.tile([128, MT], f32)
            bk = w1.tile([128, MT], f32)
            cos2 = w1.tile([128, S], f32)
            sin2 = w1.tile([128, S], f32)
            prt = w1.tile([128, 128], f16)
            wo = persist.tile([128, MT, DIM], f16)
            maskb = persist.tile([128, ST], f32)
            mask01 = persist.tile([128, ST], f32)
            # DMA order = first-consumer order: the prologue's K0/Q0 chains
            # need wk/wq row-tile 0 and the low kt slices of xT first, so
            # those are issued first (on separate engine queues so chunks
            # stream concurrently); bulk weight tails follow.
            nc.scalar.dma_start(out=wk[:, 0], in_=wk_d.ap()[:, 0])
            nc.gpsimd.dma_start(out=wq[:, 0], in_=wq_d.ap()[:, 0])
            for j in range(4):
                nc.sync.dma_start(out=xT[:, 2 * j:2 * j + 2],
                                  in_=xT_d.ap()[:, 2 * j:2 * j + 2])
            for t, d in [(bq, bq_d), (bk, bk_d),
                         (cos2, cos_d), (sin2, sin_d), (prt, prt_d),
                         (maskb, maskb_d), (mask01, mask01_d)]:
                nc.gpsimd.dma_start(out=t[:], in_=d.ap())
            nc.scalar.dma_start(out=wv[:], in_=wv_d.ap())
            nc.gpsimd.dma_start(out=wk[:, 1:4], in_=wk_d.ap()[:, 1:4])
            nc.gpsimd.dma_start(out=wq[:, 1:4], in_=wq_d.ap()[:, 1:4])
            ones_f = w1.tile([128, S], f32)
            nc.vector.memset(ones_f[:], 1.0)

            qT = persist.tile([128, MT, S], f16)
            kT = persist.tile([128, MT, S], f16)
            # V with an all-ones 65th channel: the PV matmul emits the
            # softmax rowsum at PSUM partition 64 for free (M=65);
            # whole-tile memset (strided column memsets misfire), V spills
            # overwrite channels 0-63, leaving channel 64 == 1.0
            vv = persist.tile([128, ST, HSH, 72], f16)
            nc.vector.memset(vv[:], 1.0)
            ones4 = persist.tile([97, HEAD_DIM], f16)
            nc.vector.tensor_copy(ones4[:], ones_f[0:97, 0:HEAD_DIM])

            # ---- phases 1+2: projections zippered into attention ----
            # K0/Q0/V run as a prologue; each attention (mt, c2) pass's
            # inner loop then carries later projection matmuls so PE fills
            # the gaps while ACT streams exps.
            attU = persist.tile([128, MT, S], f16)
            # rowsum reciprocals at partitions 32*(hh*2+c2)
            recq = persist.tile([97, MT, 512], f16)
            rs_sb = persist.tile([97, MT, 512], f32)
            recf = persist.tile([97, MT, 512], f32)
            recd = nc.dram_tensor("recd", [97, MT, 512], f16)

            with tc.tile_pool(name="p1ps", bufs=2, space="PSUM") as p1ps, \
                 tc.tile_pool(name="p1sb", bufs=3) as p1sb, \
                 tc.tile_pool(name="p2r", bufs=2) as p2r:

                def rope_apply(dst, b, c2, ps, pppool):
                    # row-tile 0 only: RoPE on the first 64 flat channels
                    # (rows 64-127 and the hg=1 core get identity via
                    # cos=1/sin=0 from the host).
                    sl = slice(c2 * 512, (c2 + 1) * 512)
                    sinp = p1sb.tile([128, 512], f16, tag="sinp", name="sinp")
                    nc.vector.scalar_tensor_tensor(
                        sinp[:], ps[:], b[:, 0:1],
                        sin2[:, sl], op0=OP.add, op1=OP.mult)
                    cosp = p1sb.tile([128, 512], f32, tag="cosp", name="cosp")
                    nc.vector.scalar_tensor_tensor(
                        cosp[:], ps[:], b[:, 0:1],
                        cos2[:, sl], op0=OP.add, op1=OP.mult)
                    pp = pppool.tile([128, 512], f32, tag="pp", name="pp")
                    nc.tensor.matmul(out=pp[:], lhsT=prt[:], rhs=sinp[:],
                                     start=True, stop=True)
                    nc.vector.tensor_tensor(
                        dst[:, 0, sl], cosp[:], pp[:], op=OP.add)

                def proj_v(st, pool):
                    # bv is folded in on the host: softmax weights sum to 1,
                    # so attn-out = attU/denom + bv exactly; bv@Wo joins bo.
                    ps = pool.tile([128, DSH], f32, tag="vps", name="ps")
                    for kt in range(KT):
                        nc.tensor.matmul(
                            out=ps[:],
                            lhsT=xT[:, kt, st * 128:(st + 1) * 128],
                            rhs=wv[:, kt, :],
                            start=(kt == 0), stop=(kt == KT - 1))
                    # spill on DVE: ACT's FIFO must stay clear so the first
                    # attention exps aren't queued behind these copies
                    nc.vector.tensor_copy(
                        vv[:, st, :, 0:HEAD_DIM],
                        ps[:].rearrange("p (h d) -> p h d", h=HSH))

                def proj_gen(mt):
                    # generator: one projection matmul per next(); bias-add
                    # epilogue rides with each group's final matmul.
                    for dst, w, b in ((kT, wk, bk), (qT, wq, bq)):
                        for c2 in range(2):
                            sl = slice(c2 * 512, (c2 + 1) * 512)
                            ps = p1ps.tile([128, 512], f32, tag="ps",
                                           name="ps")
                            for kt in range(KT):
                                nc.tensor.matmul(
                                    out=ps[:], lhsT=w[:, mt, kt, :],
                                    rhs=xT[:, kt, sl],
                                    start=(kt == 0), stop=(kt == KT - 1))
                                if kt == KT - 1:
                                    nc.vector.tensor_scalar(
                                        dst[:, mt, sl], ps[:], b[:, mt:mt + 1],
                                        None, op0=OP.add)
                                yield
                    while True:
                        yield

                # prologue: row-tile 0 projections + all of V, with a
                # deep PSUM pool (banks are free until the attention pools
                # open); all 32 K0/Q0 matmuls run dense, then the RoPE
                # chains consume the held PSUM tiles while V streams.
                with tc.tile_pool(name="p1pp", bufs=1,
                                  space="PSUM") as p1pp, \
                     tc.tile_pool(name="vps", bufs=5,
                                  space="PSUM") as vps:
                    kq_ps = []
                    for dst, w, b in ((kT, wk, bk), (qT, wq, bq)):
                        for c2 in range(2):
                            sl = slice(c2 * 512, (c2 + 1) * 512)
                            ps = vps.tile([128, 512], f32, tag="vps",
                                          name="ps")
                            for kt in range(KT):
                                nc.tensor.matmul(
                                    out=ps[:], lhsT=w[:, 0, kt, :],
                                    rhs=xT[:, kt, sl],
                                    start=(kt == 0), stop=(kt == KT - 1))
                            kq_ps.append((dst, b, c2, ps))
                    proj_v(0, vps)
                    for dst, b, c2, ps in kq_ps:
                        rope_apply(dst, b, c2, ps, p1pp)
                    for st in range(1, ST):
                        proj_v(st, vps)

                def normalize(mt, c2):
                    # DRAM bounce on the gpsimd DMA queue partition-
                    # broadcasts each head's 1/rowsum row; the scale runs on
                    # the otherwise-idle GpSimd engine to keep DVE clear.
                    qsl = slice(c2 * 512, (c2 + 1) * 512)
                    nc.gpsimd.dma_start(
                        out=recd.ap()[32 * c2:32 * c2 + 65:64, mt, :],
                        in_=recq[32 * c2:32 * c2 + 65:64, mt, :])
                    for hh in range(2):
                        ph = hh * 64
                        r = 32 * (hh * 2 + c2)
                        rb = p2r.tile([128, 512], f32, tag="rb", name="rb")
                        nc.gpsimd.dma_start(
                            out=rb[ph:ph + 64],
                            in_=recd.ap()[r:r + 1,
                                          mt, :].partition_broadcast(HEAD_DIM))
                        nc.gpsimd.tensor_tensor(
                            attU[ph:ph + 64, mt, qsl],
                            attU[ph:ph + 64, mt, qsl],
                            rb[ph:ph + 64], op=OP.mult)

                with tc.tile_pool(name="p2sc", bufs=2, space="PSUM") as p2sc, \
                     tc.tile_pool(name="p2at", bufs=1, space="PSUM") as p2at, \
                     tc.tile_pool(name="p2sb", bufs=2) as p2sb:

                    def emit_scores(mt, kt, c2):
                        # one [128,1024] tile per (q-half, key-chunk):
                        # h0 -> bank-half 0:512, h1 -> 512:1024 (adjacent
                        # row-group pair -> concurrent); both heads share
                        # the per-key exp bias so ONE exp covers the tile.
                        qsl = slice(c2 * 512, (c2 + 1) * 512)
                        sch = p2sc.tile([128, S], f32, tag="scc", name="scc")
                        for hh in range(2):
                            ph = hh * 64
                            nc.tensor.matmul(
                                out=sch[:, hh * 512:hh * 512 + 512],
                                lhsT=kT[ph:ph + 64, mt,
                                        kt * 128:(kt + 1) * 128],
                                rhs=qT[ph:ph + 64, mt, qsl],
                                start=True, stop=True,
                                tile_position=(ph, 0))
                        return sch

                    # q-halves run sequentially (16 key-chunk iterations per
                    # row-tile): DoubleRow PV must write PSUM partition 0, so
                    # only one q-half's pair of [65,512] accumulators fits
                    # alongside the double-buffered score tiles.
                    # nonzero through the last iterations so the PE has
                    # filler while it waits on the final exps of a pass
                    nproj_it = (3, 3, 3, 3, 2, 2, 2, 2, 2, 2, 2, 2, 1, 1, 1, 1)
                    passes = [(mt, c2) for mt in range(MT) for c2 in range(2)]
                    # wo is phase-3-only: its 1MB DMA is issued here so it
                    # does not compete with x/wq/wk/wv during the
                    # bandwidth-bound prologue window
                    nc.scalar.dma_start(out=wo[:], in_=wo_d.ap())
                    pending_recip = []
                    sch_next = emit_scores(0, 0, 0)
                    for pi, (mt, c2) in enumerate(passes):
                        if c2 == 0:
                            gen = proj_gen(mt + 1) if mt + 1 < MT else iter(
                                lambda: None, 0)  # infinite no-op iterator
                        if True:
                            qsl = slice(c2 * 512, (c2 + 1) * 512)
                            at = {hh: p2at.tile([65, 512], f32,
                                                name=f"at{hh}", tag=f"at{hh}")
                                  for hh in range(2)}
                            sch = sch_next
                            pt2 = None
                            for kt in range(ST):
                                pt2 = p2sb.tile([128, S], f16,
                                                tag="ptc", name="ptc")
                                nc.scalar.activation(
                                    pt2[:], sch[:], AF.Exp,
                                    bias=maskb[:, kt:kt + 1], scale=0.125)
                                first, last = (kt == 0), (kt == ST - 1)
                                if kt == 1 and pending_recip:
                                    # previous pass's reciprocal chain runs
                                    # here so it never gates the DVE at the
                                    # pass boundary; its consumer (the kt==2
                                    # DMA-bounce normalize) is still ahead
                                    pr = pending_recip.pop(0)
                                    nc.vector.reciprocal_approx_fast(
                                        recf[:, pr, :], rs_sb[:, pr, :])
                                    nc.vector.tensor_copy(recq[:, pr, :],
                                                          recf[:, pr, :])
                                if not last:
                                    sch = emit_scores(mt, kt + 1, c2)
                                elif pi + 1 < len(passes):
                                    # cross-pass prefetch: the next pass's
                                    # first score tile, so ACT rolls into
                                    # its exps with no pipeline drain
                                    sch_next = emit_scores(
                                        passes[pi + 1][0], 0,
                                        passes[pi + 1][1])
                                for _ in range(nproj_it[c2 * ST + kt]):
                                    next(gen)
                                for hh in range(2):  # PV+rowsum (M=65)
                                    nc.tensor.matmul(
                                        out=at[hh][:],
                                        lhsT=vv[:, kt, mt * 2 + hh, 0:65],
                                        rhs=pt2[:, hh * 512:hh * 512 + 512],
                                        start=first, stop=last)
                                if kt == 2 and c2 == 0 and mt > 0:
                                    normalize(mt - 1, 1)
                                if kt == 2 and c2 == 1:
                                    normalize(mt, 0)
                            # epilogue for this (mt, c2): stage the PSUM
                            # rowsum rows to SBUF (the custom-DVE recip
                            # mangles offset single-partition PSUM reads),
                            # one full-height reciprocal (junk partitions
                            # between the used rows are never read), then
                            # spill the attn columns
                            # rowsum staging + attn spills first -- they are
                            # the last readers of the PSUM accumulators, so
                            # the next pass's PV chain unblocks as soon as
                            # they retire; the reciprocal chain (only needed
                            # at the next pass's kt==2 bounce) runs after.
                            last_half = (mt == MT - 1 and c2 == 1)
                            for hh in range(2):
                                r = 32 * (hh * 2 + c2)
                                ph = hh * 64
                                nc.vector.tensor_copy(
                                    rs_sb[r:r + 1, mt, :], at[hh][64:65, :])
                                # one spill rides ACT when the следующий pass
                                # still has exp slack (mt0-2), halving the
                                # DVE work that gates PSUM reuse at the
                                # pass boundary
                                on_act = ((last_half and hh == 0) or
                                          (not last_half and hh == 1 and
                                           pi < 5))
                                if on_act:
                                    nc.scalar.activation(
                                        attU[ph:ph + 64, mt, qsl],
                                        at[hh][0:64, :], AF.Copy)
                                else:
                                    nc.vector.tensor_copy(
                                        attU[ph:ph + 64, mt, qsl],
                                        at[hh][0:64, :])
                            if last_half:
                                nc.vector.reciprocal_approx_fast(
                                    recf[:, mt, :], rs_sb[:, mt, :])
                                nc.vector.tensor_copy(recq[:, mt, :],
                                                      recf[:, mt, :])
                            else:
                                pending_recip.append(mt)

        # ---- phase 3: output projection -------------------------------
        # bo (and bv@Wo) are added on the host, so each (qt, c2) tile is a
        # bare 4-matmul accumulation chain; f16 output halves the out DMA.
        with tc.tile_pool(name="p3ps", bufs=8, space="PSUM") as p3ps, \
             tc.tile_pool(name="p3sb", bufs=3) as p3sb:
            # qt 0-3 read only attU columns 0-511 (normalized during the
            # c2=1 pass via the DMA bounce), so their chains are emitted
            # FIRST; the c2=1-half broadcast normalize runs on DVE under
            # them, unblocking qt 4-7.
            def final_norm():
                for hh in range(2):
                    ph = hh * 64
                    r = 32 * (hh * 2 + 1)
                    rbps = p3ps.tile([HEAD_DIM, 512], f32, tag="ps3",
                                     name="rbps")
                    nc.tensor.matmul(
                        out=rbps[:], lhsT=ones4[r:r + 1, :],
                        rhs=recq[r:r + 1, MT - 1, :],
                        start=True, stop=True, tile_position=(r, 0))
                    nc.vector.tensor_tensor(
                        attU[ph:ph + 64, MT - 1, 512:1024],
                        attU[ph:ph + 64, MT - 1, 512:1024],
                        rbps[:], op=OP.mult)
            dmaq = [nc.sync, nc.scalar, nc.gpsimd]
            for qt in list(range(4)) + ["norm"] + list(range(4, ST)):
                if qt == "norm":
                    final_norm()
                    continue
                ob = p3sb.tile([128, DIM], f16, tag="ob")
                for c2 in range(DIM // 512):
                    nsl = slice(c2 * 512, (c2 + 1) * 512)
                    ps = p3ps.tile([128, 512], f32, tag="ps3", name="ps3")
                    for mt in range(MT):
                        nc.tensor.matmul(
                            out=ps[:],
                            lhsT=attU[:, mt, qt * 128:(qt + 1) * 128],
                            rhs=wo[:, mt, nsl],
                            start=(mt == 0), stop=(mt == MT - 1))
                    # masked-row zeroing fused into the PSUM->SBUF move, on
                    # ACT (idle in this phase) to keep DVE off the path
                    nc.scalar.activation(
                        ob[:, nsl], ps[:], AF.Copy,
                        scale=mask01[:, qt:qt + 1])
                    dmaq[(qt * 2 + c2) % 3].dma_start(
                        out=out_d.ap()[qt * 128:(qt + 1) * 128, nsl],
                        in_=ob[:, nsl])

    nc.compile()
    return nc


def _get_nc():
    if "nc" not in _CACHE:
        _CACHE["nc"] = _build()
    return _CACHE["nc"]


def _prep_inputs(x, mask, freqs, Wq, bq, Wk, bk, Wv, bv, Wo, bo):
    f = np.asarray(freqs, np.float32)[0]              # [S, HEAD_DIM]
    # reference rotates only the first rot_dim=64 channels of the FLAT
    # inner dim -> rows 0-63 of row-tile 0 on the hg=0 core; everything
    # else is identity (cos=1, sin=0).
    cos2 = np.ones((128, S), np.float32)
    sin2 = np.zeros((128, S), np.float32)
    cos2[0:HEAD_DIM] = np.cos(f.T)
    sin2[0:HEAD_DIM] = np.sin(f.T)
    ident = np.ones((128, S), np.float32)
    identz = np.zeros((128, S), np.float32)

    prt = np.zeros((128, 128), np.float16)            # P_rot^T
    i = np.arange(0, 128, 2)
    prt[i + 1, i] = -1.0                              # P_rot[2i, 2i+1] = -1
    prt[i, i + 1] = 1.0                               # P_rot[2i+1, 2i] = +1

    def lhsT_w(w):                                    # [DIM, DSH] -> lhsT tiles
        # mt-major so the row-tile-0 slice is one contiguous DMA chunk
        return np.ascontiguousarray(
            w.reshape(KT, 128, MT, 128).transpose(1, 2, 0, 3)).astype(np.float16)

    def col(b):                                       # [DSH] -> [128, MT]
        return np.ascontiguousarray(b.reshape(MT, 128).T.astype(np.float32))

    in_maps = []
    for b in range(B):
        xT = np.ascontiguousarray(
            np.asarray(x[b], np.float32).T.reshape(KT, 128, S)
            .transpose(1, 0, 2)).astype(np.float16)
        m = np.asarray(mask[b])
        maskb = np.ascontiguousarray(
            np.where(m, 0.0, MASK_NEG).astype(np.float32).reshape(ST, 128).T)
        mask01 = np.ascontiguousarray(
            m.astype(np.float32).reshape(ST, 128).T)
        for hg in range(HG):
            dsl = slice(hg * DSH, (hg + 1) * DSH)
            in_maps.append({
                "xT": xT,
                "wq": lhsT_w(np.asarray(Wq, np.float32)[:, dsl]),
                "wk": lhsT_w(np.asarray(Wk, np.float32)[:, dsl]),
                "wv": np.ascontiguousarray(
                    np.asarray(Wv, np.float32)[:, dsl]
                    .reshape(KT, 128, DSH).transpose(1, 0, 2)).astype(np.float16),
                "wo": np.ascontiguousarray(
                    np.asarray(Wo, np.float32)[dsl, :]
                    .reshape(MT, 128, DIM).transpose(1, 0, 2)).astype(np.float16),
                "bq": col(np.asarray(bq, np.float32)[dsl]),
                "bk": col(np.asarray(bk, np.float32)[dsl]),
                "cos2": cos2 if hg == 0 else ident,
                "sin2": sin2 if hg == 0 else identz,
                "prt": prt,
                "maskb": maskb, "mask01": mask01,
            })
    return in_maps


def run(trace=False, **inputs):
    from concourse import bass_utils
    if trace:
        _install_ntff_hook()
    nc = _get_nc()
    in_maps = _prep_inputs(**inputs)
    res = bass_utils.run_bass_kernel_spmd(
        nc, in_maps, core_ids=list(range(NCORES)), trace=trace)
    # host epilogue: sum the two head-group partials and add the biases the
    # device never sees (bo, plus bv@Wo -- softmax weights sum to 1, so the
    # v-bias shifts every attn-out row by exactly bv); masked rows are 0.
    bias = (np.asarray(inputs["bo"], np.float32)
            + np.asarray(inputs["bv"], np.float32)
            @ np.asarray(inputs["Wo"], np.float32))
    mask = np.asarray(inputs["mask"])
    out = np.empty((B, S, DIM), np.float32)
    for b in range(B):
        s = (res.results[2 * b]["out"].astype(np.float32)
             + res.results[2 * b + 1]["out"].astype(np.float32) + bias)
        out[b] = np.where(mask[b][:, None], s, 0.0)
    return out, res


def kernel(**inputs):
    out, _ = run(trace=False, **inputs)
    return out


def _install_ntff_hook():
    """Register the axon NTFF profiling hook missing from the antenv stub."""
    import sys, types
    try:
        import antenv.axon_hooks  # noqa: F401
        return
    except ImportError:
        pass
    from trn_agent_boot.trn_boot import _ntff_profile_via_ctypes
    hook = _ntff_profile_via_ctypes('/opt/axon/libaxon_pjrt.so')
    mod = types.ModuleType('antenv.axon_hooks')
    mod.get_axon_ntff_profile_hook = lambda: hook
    mod.set_axon_ntff_profile_hook = lambda h: None
    sys.modules['antenv.axon_hooks'] = mod



# revision 52
# speedup vs baseline: 1.0383x; 1.0199x over previous
"""Trainium2 Bass kernel for nn_Attention (B=4, S=1024, DIM=1024, H=16, Dh=64).

Sharding: 8 cores = 4 batches x 2 head-groups (8 heads / 512 inner channels
each).  Each core computes q/k/v projections for its head shard, RoPE,
attention, and a partial output projection (its rows of Wo); the host sums
the two head-group partials per batch (the tensor-parallel all-reduce done
on host), adds the biases the device never sees (bo + bv@Wo -- softmax
weights sum to 1, so the v-bias shifts attn-out by exactly bv), and
re-applies the row mask.

Device dataflow (per core), matmul operands in fp16 (fp32 PSUM accumulate):
  input DMAs chunked in first-consumer order so the prologue's first
  matmuls start ~13us in while the weight tails stream ->
  Q^T,K^T = W^T @ x^T      (bias added on the PSUM->SBUF pass)
  RoPE on the first 64 flat channels only (reference rotates rot_dim=64 of
  the flat inner dim): qr = (q+b)*cos + P_rot@((q+b)*sin), P_rot on PE.
  Attention runs as 8 sequential (row-tile, q-half) passes of 8 key-chunk
  iterations each, with later row-tiles' projection matmuls zippered in:
  scores^T[k,q] = K_h @ Q_h^T   (K=64; the two heads of a pass issue on
                                 row groups 0/64 -> concurrent; the next
                                 pass's first tile is emitted at kt==7 so
                                 ACT never drains)
  P^T = exp(scores^T/8 + maskbias[k])  (ACT, one op per pass-iteration;
                                        key mask folded into the exp bias)
  attn^T[c,q] = V_aug^T @ P^T   (V_aug carries an all-ones 65th channel, so
                                 the matmul emits the softmax rowsum at PSUM
                                 partition 64 for free -- no separate
                                 rowsum matmuls)
  epilogue per pass: rowsum rows staged to SBUF (the custom-DVE reciprocal
  mangles offset single-partition PSUM reads) and attn spilled first (they
  gate the next pass's PSUM reuse), then reciprocal -> f16; normalization
  multiplies ride later passes via a DRAM-bounce partition-broadcast on
  the GpSimd queue; only the last q-half normalizes via PE broadcast.
  Phase 3: out[q,:] = attn^T.T @ Wo_shard as bare 4-matmul chains (biases
  on host), masked rows zeroed on the ACT PSUM->SBUF copy, f16 output DMA
  spread over three queues; qt 0-3 are emitted before the final-normalize
  broadcast so they overlap its DVE mults.
"""

import numpy as np

B, S, DIM, HEADS, HEAD_DIM = 4, 1024, 1024, 16, 64
INNER = HEADS * HEAD_DIM
HG = 2                      # head groups (tensor-parallel shards)
DSH = INNER // HG           # 512 inner channels per core
HSH = HEADS // HG           # 8 heads per core
NCORES = B * HG
KT = DIM // 128             # 8 contraction tiles
MT = DSH // 128             # 4 output row tiles for Q^T/K^T
ST = S // 128               # 8 seq tiles
MASK_NEG = -80.0

_CACHE = {}


def _build():
    import concourse.tile as tile
    from concourse import bacc, mybir

    f32 = mybir.dt.float32
    f16 = mybir.dt.float16
    f8 = mybir.dt.float8e4
    AF = mybir.ActivationFunctionType
    OP = mybir.AluOpType
    DR = mybir.MatmulPerfMode.DoubleRow

    nc = bacc.Bacc("TRN2", target_bir_lowering=False, debug=False)

    xT_d = nc.dram_tensor("xT", [128, KT, S], f16, kind="ExternalInput")
    wq_d = nc.dram_tensor("wq", [128, MT, KT, 128], f16, kind="ExternalInput")
    wk_d = nc.dram_tensor("wk", [128, MT, KT, 128], f16, kind="ExternalInput")
    wv_d = nc.dram_tensor("wv", [128, KT, DSH], f16, kind="ExternalInput")
    wo_d = nc.dram_tensor("wo", [128, MT, DIM], f16, kind="ExternalInput")
    bq_d = nc.dram_tensor("bq", [128, MT], f32, kind="ExternalInput")
    bk_d = nc.dram_tensor("bk", [128, MT], f32, kind="ExternalInput")
    cos_d = nc.dram_tensor("cos2", [128, S], f32, kind="ExternalInput")
    sin_d = nc.dram_tensor("sin2", [128, S], f32, kind="ExternalInput")
    prt_d = nc.dram_tensor("prt", [128, 128], f16, kind="ExternalInput")
    maskb_d = nc.dram_tensor("maskb", [128, ST], f32, kind="ExternalInput")
    mask01_d = nc.dram_tensor("mask01", [128, ST], f32, kind="ExternalInput")
    out_d = nc.dram_tensor("out", [S, DIM], f16, kind="ExternalOutput")

    with tile.TileContext(nc) as tc, \
         tc.tile_pool(name="persist", bufs=1) as persist:
        with tc.tile_pool(name="w1", bufs=1) as w1:
            # phase-1-only constants
            xT = w1.tile([128, KT, S], f16)
            wq = w1.tile([128, MT, KT, 128], f16)
            wk = w1.tile([128, MT, KT, 128], f16)
            wv = w1.tile([128, KT, DSH], f16)
            bq = w1.tile([128, MT], f32)
            bk = w1.tile([128, MT], f32)
            cos2 = w1.tile([128, S], f32)
            sin2 = w1.tile([128, S], f32)
            prt = w1.tile([128, 128], f16)
            wo = persist.tile([128, MT, DIM], f16)
            maskb = persist.tile([128, ST], f32)
            mask01 = persist.tile([128, ST], f32)
            # DMA order = first-consumer order: the prologue's K0/Q0 chains
            # need wk/wq row-tile 0 and the low kt slices of xT first, so
            # those are issued first (on separate engine queues so chunks
            # stream concurrently); bulk weight tails follow.
            nc.scalar.dma_start(out=wk[:, 0], in_=wk_d.ap()[:, 0])
            nc.gpsimd.dma_start(out=wq[:, 0], in_=wq_d.ap()[:, 0])
            for j in range(4):
                nc.sync.dma_start(out=xT[:, 2 * j:2 * j + 2],
                                  in_=xT_d.ap()[:, 2 * j:2 * j + 2])
            for t, d in [(bq, bq_d), (bk, bk_d),
                         (cos2, cos_d), (sin2, sin_d), (prt, prt_d),
                         (maskb, maskb_d), (mask01, mask01_d)]:
                nc.gpsimd.dma_start(out=t[:], in_=d.ap())
            nc.scalar.dma_start(out=wv[:], in_=wv_d.ap())
            nc.gpsimd.dma_start(out=wk[:, 1:4], in_=wk_d.ap()[:, 1:4])
            nc.gpsimd.dma_start(out=wq[:, 1:4], in_=wq_d.ap()[:, 1:4])
            ones_f = w1.tile([128, S], f32)
            nc.vector.memset(ones_f[:], 1.0)

            qT = persist.tile([128, MT, S], f16)
            kT = persist.tile([128, MT, S], f16)
            # V with an all-ones 65th channel: the PV matmul emits the
            # softmax rowsum at PSUM partition 64 for free (M=65);
            # whole-tile memset (strided column memsets misfire), V spills
            # overwrite channels 0-63, leaving channel 64 == 1.0
            vv = persist.tile([128, ST, HSH, 72], f16)
            nc.vector.memset(vv[:], 1.0)
            ones4 = persist.tile([97, HEAD_DIM], f16)
            nc.vector.tensor_copy(ones4[:], ones_f[0:97, 0:HEAD_DIM])

            # ---- phases 1+2: projections zippered into attention ----
            # K0/Q0/V run as a prologue; each attention (mt, c2) pass's
            # inner loop then carries later projection matmuls so PE fills
            # the gaps while ACT streams exps.
            attU = persist.tile([128, MT, S], f16)
            # rowsum reciprocals at partitions 32*(hh*2+c2)
            recq = persist.tile([97, MT, 512], f16)
            rs_sb = persist.tile([97, MT, 512], f32)
            recf = persist.tile([97, MT, 512], f32)
            recd = nc.dram_tensor("recd", [97, MT, 512], f16)

            with tc.tile_pool(name="p1ps", bufs=2, space="PSUM") as p1ps, \
                 tc.tile_pool(name="p1sb", bufs=3) as p1sb, \
                 tc.tile_pool(name="p2r", bufs=2) as p2r:

                def rope_apply(dst, b, c2, ps, pppool):
                    # row-tile 0 only: RoPE on the first 64 flat channels
                    # (rows 64-127 and the hg=1 core get identity via
                    # cos=1/sin=0 from the host).
                    sl = slice(c2 * 512, (c2 + 1) * 512)
                    sinp = p1sb.tile([128, 512], f16, tag="sinp", name="sinp")
                    nc.vector.scalar_tensor_tensor(
                        sinp[:], ps[:], b[:, 0:1],
                        sin2[:, sl], op0=OP.add, op1=OP.mult)
                    cosp = p1sb.tile([128, 512], f32, tag="cosp", name="cosp")
                    nc.vector.scalar_tensor_tensor(
                        cosp[:], ps[:], b[:, 0:1],
                        cos2[:, sl], op0=OP.add, op1=OP.mult)
                    pp = pppool.tile([128, 512], f32, tag="pp", name="pp")
                    nc.tensor.matmul(out=pp[:], lhsT=prt[:], rhs=sinp[:],
                                     start=True, stop=True)
                    nc.vector.tensor_tensor(
                        dst[:, 0, sl], cosp[:], pp[:], op=OP.add)

                def proj_v(st, pool):
                    # bv is folded in on the host: softmax weights sum to 1,
                    # so attn-out = attU/denom + bv exactly; bv@Wo joins bo.
                    ps = pool.tile([128, DSH], f32, tag="vps", name="ps")
                    for kt in range(KT):
                        nc.tensor.matmul(
                            out=ps[:],
                            lhsT=xT[:, kt, st * 128:(st + 1) * 128],
                            rhs=wv[:, kt, :],
                            start=(kt == 0), stop=(kt == KT - 1))
                    # spill on DVE: ACT's FIFO must stay clear so the first
                    # attention exps aren't queued behind these copies
                    nc.vector.tensor_copy(
                        vv[:, st, :, 0:HEAD_DIM],
                        ps[:].rearrange("p (h d) -> p h d", h=HSH))

                def proj_gen(mt):
                    # generator: one projection matmul per next(); bias-add
                    # epilogue rides with each group's final matmul.
                    for dst, w, b in ((kT, wk, bk), (qT, wq, bq)):
                        for c2 in range(2):
                            sl = slice(c2 * 512, (c2 + 1) * 512)
                            ps = p1ps.tile([128, 512], f32, tag="ps",
                                           name="ps")
                            for kt in range(KT):
                                nc.tensor.matmul(
                                    out=ps[:], lhsT=w[:, mt, kt, :],
                                    rhs=xT[:, kt, sl],
                                    start=(kt == 0), stop=(kt == KT - 1))
                                if kt == KT - 1:
                                    nc.vector.tensor_scalar(
                                        dst[:, mt, sl], ps[:], b[:, mt:mt + 1],
                                        None, op0=OP.add)
                                yield
                    while True:
                        yield

                # prologue: row-tile 0 projections + all of V, with a
                # deep PSUM pool (banks are free until the attention pools
                # open); all 32 K0/Q0 matmuls run dense, then the RoPE
                # chains consume the held PSUM tiles while V streams.
                with tc.tile_pool(name="p1pp", bufs=1,
                                  space="PSUM") as p1pp, \
                     tc.tile_pool(name="vps", bufs=5,
                                  space="PSUM") as vps:
                    kq_ps = []
                    for dst, w, b in ((kT, wk, bk), (qT, wq, bq)):
                        for c2 in range(2):
                            sl = slice(c2 * 512, (c2 + 1) * 512)
                            ps = vps.tile([128, 512], f32, tag="vps",
                                          name="ps")
                            for kt in range(KT):
                                nc.tensor.matmul(
                                    out=ps[:], lhsT=w[:, 0, kt, :],
                                    rhs=xT[:, kt, sl],
                                    start=(kt == 0), stop=(kt == KT - 1))
                            kq_ps.append((dst, b, c2, ps))
                    proj_v(0, vps)
                    for dst, b, c2, ps in kq_ps:
                        rope_apply(dst, b, c2, ps, p1pp)
                    for st in range(1, ST):
                        proj_v(st, vps)

                def normalize(mt, c2):
                    # DRAM bounce on the gpsimd DMA queue partition-
                    # broadcasts each head's 1/rowsum row; the scale runs on
                    # the otherwise-idle GpSimd engine to keep DVE clear.
                    qsl = slice(c2 * 512, (c2 + 1) * 512)
                    nc.gpsimd.dma_start(
                        out=recd.ap()[32 * c2:32 * c2 + 65:64, mt, :],
                        in_=recq[32 * c2:32 * c2 + 65:64, mt, :])
                    for hh in range(2):
                        ph = hh * 64
                        r = 32 * (hh * 2 + c2)
                        rb = p2r.tile([128, 512], f32, tag="rb", name="rb")
                        nc.gpsimd.dma_start(
                            out=rb[ph:ph + 64],
                            in_=recd.ap()[r:r + 1,
                                          mt, :].partition_broadcast(HEAD_DIM))
                        nc.gpsimd.tensor_tensor(
                            attU[ph:ph + 64, mt, qsl],
                            attU[ph:ph + 64, mt, qsl],
                            rb[ph:ph + 64], op=OP.mult)

                with tc.tile_pool(name="p2sc", bufs=2, space="PSUM") as p2sc, \
                     tc.tile_pool(name="p2at", bufs=1, space="PSUM") as p2at, \
                     tc.tile_pool(name="p2sb", bufs=3) as p2sb:

                    def emit_scores(mt, kt, c2):
                        # one [128,1024] tile per (q-half, key-chunk):
                        # h0 -> bank-half 0:512, h1 -> 512:1024 (adjacent
                        # row-group pair -> concurrent); both heads share
                        # the per-key exp bias so ONE exp covers the tile.
                        qsl = slice(c2 * 512, (c2 + 1) * 512)
                        sch = p2sc.tile([128, S], f32, tag="scc", name="scc")
                        for hh in range(2):
                            ph = hh * 64
                            nc.tensor.matmul(
                                out=sch[:, hh * 512:hh * 512 + 512],
                                lhsT=kT[ph:ph + 64, mt,
                                        kt * 128:(kt + 1) * 128],
                                rhs=qT[ph:ph + 64, mt, qsl],
                                start=True, stop=True,
                                tile_position=(ph, 0))
                        return sch

                    # q-halves run sequentially (16 key-chunk iterations per
                    # row-tile): DoubleRow PV must write PSUM partition 0, so
                    # only one q-half's pair of [65,512] accumulators fits
                    # alongside the double-buffered score tiles.
                    # nonzero through the last iterations so the PE has
                    # filler while it waits on the final exps of a pass
                    nproj_it = (3, 3, 3, 3, 2, 2, 2, 2, 2, 2, 2, 2, 1, 1, 1, 1)
                    passes = [(mt, c2) for mt in range(MT) for c2 in range(2)]
                    # wo is phase-3-only: its 1MB DMA is issued here so it
                    # does not compete with x/wq/wk/wv during the
                    # bandwidth-bound prologue window
                    nc.scalar.dma_start(out=wo[:], in_=wo_d.ap())
                    pending_recip = []
                    sch_next = emit_scores(0, 0, 0)
                    for pi, (mt, c2) in enumerate(passes):
                        if c2 == 0:
                            gen = proj_gen(mt + 1) if mt + 1 < MT else iter(
                                lambda: None, 0)  # infinite no-op iterator
                        if True:
                            qsl = slice(c2 * 512, (c2 + 1) * 512)
                            at = {hh: p2at.tile([65, 512], f32,
                                                name=f"at{hh}", tag=f"at{hh}")
                                  for hh in range(2)}
                            sch = sch_next
                            pt2 = None
                            for kt in range(ST):
                                pt2 = p2sb.tile([128, S], f16,
                                                tag="ptc", name="ptc")
                                nc.scalar.activation(
                                    pt2[:], sch[:], AF.Exp,
                                    bias=maskb[:, kt:kt + 1], scale=0.125)
                                first, last = (kt == 0), (kt == ST - 1)
                                if kt == 1 and pending_recip:
                                    # previous pass's reciprocal chain runs
                                    # here so it never gates the DVE at the
                                    # pass boundary; its consumer (the kt==2
                                    # DMA-bounce normalize) is still ahead
                                    pr = pending_recip.pop(0)
                                    nc.vector.reciprocal_approx_fast(
                                        recf[:, pr, :], rs_sb[:, pr, :])
                                    nc.vector.tensor_copy(recq[:, pr, :],
                                                          recf[:, pr, :])
                                if not last:
                                    sch = emit_scores(mt, kt + 1, c2)
                                elif pi + 1 < len(passes):
                                    # cross-pass prefetch: the next pass's
                                    # first score tile, so ACT rolls into
                                    # its exps with no pipeline drain
                                    sch_next = emit_scores(
                                        passes[pi + 1][0], 0,
                                        passes[pi + 1][1])
                                for _ in range(nproj_it[c2 * ST + kt]):
                                    next(gen)
                                for hh in range(2):  # PV+rowsum (M=65)
                                    nc.tensor.matmul(
                                        out=at[hh][:],
                                        lhsT=vv[:, kt, mt * 2 + hh, 0:65],
                                        rhs=pt2[:, hh * 512:hh * 512 + 512],
                                        start=first, stop=last)
                                if kt == 2 and c2 == 0 and mt > 0:
                                    normalize(mt - 1, 1)
                                if kt == 2 and c2 == 1:
                                    normalize(mt, 0)
                            # epilogue for this (mt, c2): stage the PSUM
                            # rowsum rows to SBUF (the custom-DVE recip
                            # mangles offset single-partition PSUM reads),
                            # one full-height reciprocal (junk partitions
                            # between the used rows are never read), then
                            # spill the attn columns
                            # rowsum staging + attn spills first -- they are
                            # the last readers of the PSUM accumulators, so
                            # the next pass's PV chain unblocks as soon as
                            # they retire; the reciprocal chain (only needed
                            # at the next pass's kt==2 bounce) runs after.
                            last_half = (mt == MT - 1 and c2 == 1)
                            for hh in range(2):
                                r = 32 * (hh * 2 + c2)
                                ph = hh * 64
                                nc.vector.tensor_copy(
                                    rs_sb[r:r + 1, mt, :], at[hh][64:65, :])
                                # one spill rides ACT when the следующий pass
                                # still has exp slack (mt0-2), halving the
                                # DVE work that gates PSUM reuse at the
                                # pass boundary
                                on_act = ((last_half and hh == 0) or
                                          (not last_half and hh == 1 and
                                           pi < 5))
                                if on_act:
                                    nc.scalar.activation(
                                        attU[ph:ph + 64, mt, qsl],
                                        at[hh][0:64, :], AF.Copy)
                                else:
                                    nc.vector.tensor_copy(
                                        attU[ph:ph + 64, mt, qsl],
                                        at[hh][0:64, :])
                            if last_half:
                                nc.vector.reciprocal_approx_fast(
                                    recf[:, mt, :], rs_sb[:, mt, :])
                                nc.vector.tensor_copy(recq[:, mt, :],
                                                      recf[:, mt, :])
                            else:
                                pending_recip.append(mt)

        # ---- phase 3: output projection -------------------------------
        # bo (and bv@Wo) are added on the host, so each (qt, c2) tile is a
        # bare 4-matmul accumulation chain; f16 output halves the out DMA.
        with tc.tile_pool(name="p3ps", bufs=8, space="PSUM") as p3ps, \
             tc.tile_pool(name="p3sb", bufs=3) as p3sb:
            # qt 0-3 read only attU columns 0-511 (normalized during the
            # c2=1 pass via the DMA bounce), so their chains are emitted
            # FIRST; the c2=1-half broadcast normalize runs on DVE under
            # them, unblocking qt 4-7.
            def final_norm():
                for hh in range(2):
                    ph = hh * 64
                    r = 32 * (hh * 2 + 1)
                    rbps = p3ps.tile([HEAD_DIM, 512], f32, tag="ps3",
                                     name="rbps")
                    nc.tensor.matmul(
                        out=rbps[:], lhsT=ones4[r:r + 1, :],
                        rhs=recq[r:r + 1, MT - 1, :],
                        start=True, stop=True, tile_position=(r, 0))
                    nc.vector.tensor_tensor(
                        attU[ph:ph + 64, MT - 1, 512:1024],
                        attU[ph:ph + 64, MT - 1, 512:1024],
                        rbps[:], op=OP.mult)
            dmaq = [nc.sync, nc.scalar, nc.gpsimd]
            for qt in list(range(4)) + ["norm"] + list(range(4, ST)):
                if qt == "norm":
                    final_norm()
                    continue
                ob = p3sb.tile([128, DIM], f16, tag="ob")
                for c2 in range(DIM // 512):
                    nsl = slice(c2 * 512, (c2 + 1) * 512)
                    ps = p3ps.tile([128, 512], f32, tag="ps3", name="ps3")
                    for mt in range(MT):
                        nc.tensor.matmul(
                            out=ps[:],
                            lhsT=attU[:, mt, qt * 128:(qt + 1) * 128],
                            rhs=wo[:, mt, nsl],
                            start=(mt == 0), stop=(mt == MT - 1))
                    # masked-row zeroing fused into the PSUM->SBUF move, on
                    # ACT (idle in this phase) to keep DVE off the path
                    nc.scalar.activation(
                        ob[:, nsl], ps[:], AF.Copy,
                        scale=mask01[:, qt:qt + 1])
                    dmaq[(qt * 2 + c2) % 3].dma_start(
                        out=out_d.ap()[qt * 128:(qt + 1) * 128, nsl],
                        in_=ob[:, nsl])

    nc.compile()
    return nc


def _get_nc():
    if "nc" not in _CACHE:
        _CACHE["nc"] = _build()
    return _CACHE["nc"]


def _prep_inputs(x, mask, freqs, Wq, bq, Wk, bk, Wv, bv, Wo, bo):
    f = np.asarray(freqs, np.float32)[0]              # [S, HEAD_DIM]
    # reference rotates only the first rot_dim=64 channels of the FLAT
    # inner dim -> rows 0-63 of row-tile 0 on the hg=0 core; everything
    # else is identity (cos=1, sin=0).
    cos2 = np.ones((128, S), np.float32)
    sin2 = np.zeros((128, S), np.float32)
    cos2[0:HEAD_DIM] = np.cos(f.T)
    sin2[0:HEAD_DIM] = np.sin(f.T)
    ident = np.ones((128, S), np.float32)
    identz = np.zeros((128, S), np.float32)

    prt = np.zeros((128, 128), np.float16)            # P_rot^T
    i = np.arange(0, 128, 2)
    prt[i + 1, i] = -1.0                              # P_rot[2i, 2i+1] = -1
    prt[i, i + 1] = 1.0                               # P_rot[2i+1, 2i] = +1

    def lhsT_w(w):                                    # [DIM, DSH] -> lhsT tiles
        # mt-major so the row-tile-0 slice is one contiguous DMA chunk
        return np.ascontiguousarray(
            w.reshape(KT, 128, MT, 128).transpose(1, 2, 0, 3)).astype(np.float16)

    def col(b):                                       # [DSH] -> [128, MT]
        return np.ascontiguousarray(b.reshape(MT, 128).T.astype(np.float32))

    in_maps = []
    for b in range(B):
        xT = np.ascontiguousarray(
            np.asarray(x[b], np.float32).T.reshape(KT, 128, S)
            .transpose(1, 0, 2)).astype(np.float16)
        m = np.asarray(mask[b])
        maskb = np.ascontiguousarray(
            np.where(m, 0.0, MASK_NEG).astype(np.float32).reshape(ST, 128).T)
        mask01 = np.ascontiguousarray(
            m.astype(np.float32).reshape(ST, 128).T)
        for hg in range(HG):
            dsl = slice(hg * DSH, (hg + 1) * DSH)
            in_maps.append({
                "xT": xT,
                "wq": lhsT_w(np.asarray(Wq, np.float32)[:, dsl]),
                "wk": lhsT_w(np.asarray(Wk, np.float32)[:, dsl]),
                "wv": np.ascontiguousarray(
                    np.asarray(Wv, np.float32)[:, dsl]
                    .reshape(KT, 128, DSH).transpose(1, 0, 2)).astype(np.float16),
                "wo": np.ascontiguousarray(
                    np.asarray(Wo, np.float32)[dsl, :]
                    .reshape(MT, 128, DIM).transpose(1, 0, 2)).astype(np.float16),
                "bq": col(np.asarray(bq, np.float32)[dsl]),
                "bk": col(np.asarray(bk, np.float32)[dsl]),
                "cos2": cos2 if hg == 0 else ident,
                "sin2": sin2 if hg == 0 else identz,
                "prt": prt,
                "maskb": maskb, "mask01": mask01,
            })
    return in_maps


def run(trace=False, **inputs):
    from concourse import bass_utils
    if trace:
        _install_ntff_hook()
    nc = _get_nc()
    in_maps = _prep_inputs(**inputs)
    res = bass_utils.run_bass_kernel_spmd(
        nc, in_maps, core_ids=list(range(NCORES)), trace=trace)
    # host epilogue: sum the two head-group partials and add the biases the
    # device never sees (bo, plus bv@Wo -- softmax weights sum to 1, so the
    # v-bias shifts every attn-out row by exactly bv); masked rows are 0.
    bias = (np.asarray(inputs["bo"], np.float32)
            + np.asarray(inputs["bv"], np.float32)
            @ np.asarray(inputs["Wo"], np.float32))
    mask = np.asarray(inputs["mask"])
    out = np.empty((B, S, DIM), np.float32)
    for b in range(B):
        s = (res.results[2 * b]["out"].astype(np.float32)
             + res.results[2 * b + 1]["out"].astype(np.float32) + bias)
        out[b] = np.where(mask[b][:, None], s, 0.0)
    return out, res


def kernel(**inputs):
    out, _ = run(trace=False, **inputs)
    return out


def _install_ntff_hook():
    """Register the axon NTFF profiling hook missing from the antenv stub."""
    import sys, types
    try:
        import antenv.axon_hooks  # noqa: F401
        return
    except ImportError:
        pass
    from trn_agent_boot.trn_boot import _ntff_profile_via_ctypes
    hook = _ntff_profile_via_ctypes('/opt/axon/libaxon_pjrt.so')
    mod = types.ModuleType('antenv.axon_hooks')
    mod.get_axon_ntff_profile_hook = lambda: hook
    mod.set_axon_ntff_profile_hook = lambda h: None
    sys.modules['antenv.axon_hooks'] = mod



# revision 53
# speedup vs baseline: 1.0410x; 1.0026x over previous
"""Trainium2 Bass kernel for nn_Attention (B=4, S=1024, DIM=1024, H=16, Dh=64).

Sharding: 8 cores = 4 batches x 2 head-groups (8 heads / 512 inner channels
each).  Each core computes q/k/v projections for its head shard, RoPE,
attention, and a partial output projection (its rows of Wo); the host sums
the two head-group partials per batch (the tensor-parallel all-reduce done
on host), adds the biases the device never sees (bo + bv@Wo -- softmax
weights sum to 1, so the v-bias shifts attn-out by exactly bv), and
re-applies the row mask.

Device dataflow (per core), matmul operands in fp16 (fp32 PSUM accumulate):
  input DMAs chunked in first-consumer order so the prologue's first
  matmuls start ~13us in while the weight tails stream ->
  Q^T,K^T = W^T @ x^T      (bias added on the PSUM->SBUF pass)
  RoPE on the first 64 flat channels only (reference rotates rot_dim=64 of
  the flat inner dim): qr = (q+b)*cos + P_rot@((q+b)*sin), P_rot on PE.
  Attention runs as 8 sequential (row-tile, q-half) passes of 8 key-chunk
  iterations each, with later row-tiles' projection matmuls zippered in:
  scores^T[k,q] = K_h @ Q_h^T   (K=64; the two heads of a pass issue on
                                 row groups 0/64 -> concurrent; the next
                                 pass's first tile is emitted at kt==7 so
                                 ACT never drains)
  P^T = exp(scores^T/8 + maskbias[k])  (ACT, one op per pass-iteration;
                                        key mask folded into the exp bias)
  attn^T[c,q] = V_aug^T @ P^T   (V_aug carries an all-ones 65th channel, so
                                 the matmul emits the softmax rowsum at PSUM
                                 partition 64 for free -- no separate
                                 rowsum matmuls)
  epilogue per pass: rowsum rows staged to SBUF (the custom-DVE reciprocal
  mangles offset single-partition PSUM reads) and attn spilled first (they
  gate the next pass's PSUM reuse), then reciprocal -> f16; normalization
  multiplies ride later passes via a DRAM-bounce partition-broadcast on
  the GpSimd queue; only the last q-half normalizes via PE broadcast.
  Phase 3: out[q,:] = attn^T.T @ Wo_shard as bare 4-matmul chains (biases
  on host), masked rows zeroed on the ACT PSUM->SBUF copy, f16 output DMA
  spread over three queues; qt 0-3 are emitted before the final-normalize
  broadcast so they overlap its DVE mults.
"""

import numpy as np

B, S, DIM, HEADS, HEAD_DIM = 4, 1024, 1024, 16, 64
INNER = HEADS * HEAD_DIM
HG = 2                      # head groups (tensor-parallel shards)
DSH = INNER // HG           # 512 inner channels per core
HSH = HEADS // HG           # 8 heads per core
NCORES = B * HG
KT = DIM // 128             # 8 contraction tiles
MT = DSH // 128             # 4 output row tiles for Q^T/K^T
ST = S // 128               # 8 seq tiles
MASK_NEG = -80.0

_CACHE = {}


def _build():
    import concourse.tile as tile
    from concourse import bacc, mybir

    f32 = mybir.dt.float32
    f16 = mybir.dt.float16
    f8 = mybir.dt.float8e4
    AF = mybir.ActivationFunctionType
    OP = mybir.AluOpType
    DR = mybir.MatmulPerfMode.DoubleRow

    nc = bacc.Bacc("TRN2", target_bir_lowering=False, debug=False)

    xT_d = nc.dram_tensor("xT", [128, KT, S], f16, kind="ExternalInput")
    wq_d = nc.dram_tensor("wq", [128, MT, KT, 128], f16, kind="ExternalInput")
    wk_d = nc.dram_tensor("wk", [128, MT, KT, 128], f16, kind="ExternalInput")
    wv_d = nc.dram_tensor("wv", [128, KT, DSH], f16, kind="ExternalInput")
    wo_d = nc.dram_tensor("wo", [128, MT, DIM], f16, kind="ExternalInput")
    bq_d = nc.dram_tensor("bq", [128, MT], f32, kind="ExternalInput")
    bk_d = nc.dram_tensor("bk", [128, MT], f32, kind="ExternalInput")
    cos_d = nc.dram_tensor("cos2", [128, S], f32, kind="ExternalInput")
    sin_d = nc.dram_tensor("sin2", [128, S], f32, kind="ExternalInput")
    prt_d = nc.dram_tensor("prt", [128, 128], f16, kind="ExternalInput")
    maskb_d = nc.dram_tensor("maskb", [128, ST], f32, kind="ExternalInput")
    mask01_d = nc.dram_tensor("mask01", [128, ST], f32, kind="ExternalInput")
    out_d = nc.dram_tensor("out", [S, DIM], f16, kind="ExternalOutput")

    with tile.TileContext(nc) as tc, \
         tc.tile_pool(name="persist", bufs=1) as persist:
        with tc.tile_pool(name="w1", bufs=1) as w1:
            # phase-1-only constants
            xT = w1.tile([128, KT, S], f16)
            wq = w1.tile([128, MT, KT, 128], f16)
            wk = w1.tile([128, MT, KT, 128], f16)
            wv = w1.tile([128, KT, DSH], f16)
            bq = w1.tile([128, MT], f32)
            bk = w1.tile([128, MT], f32)
            cos2 = w1.tile([128, S], f32)
            sin2 = w1.tile([128, S], f32)
            prt = w1.tile([128, 128], f16)
            wo = persist.tile([128, MT, DIM], f16)
            maskb = persist.tile([128, ST], f32)
            mask01 = persist.tile([128, ST], f32)
            # DMA order = first-consumer order: the prologue's K0/Q0 chains
            # need wk/wq row-tile 0 and the low kt slices of xT first, so
            # those are issued first (on separate engine queues so chunks
            # stream concurrently); bulk weight tails follow.
            nc.scalar.dma_start(out=wk[:, 0], in_=wk_d.ap()[:, 0])
            nc.gpsimd.dma_start(out=wq[:, 0], in_=wq_d.ap()[:, 0])
            for j in range(4):
                nc.sync.dma_start(out=xT[:, 2 * j:2 * j + 2],
                                  in_=xT_d.ap()[:, 2 * j:2 * j + 2])
            for t, d in [(bq, bq_d), (bk, bk_d),
                         (cos2, cos_d), (sin2, sin_d), (prt, prt_d),
                         (maskb, maskb_d), (mask01, mask01_d)]:
                nc.gpsimd.dma_start(out=t[:], in_=d.ap())
            nc.scalar.dma_start(out=wv[:], in_=wv_d.ap())
            nc.gpsimd.dma_start(out=wk[:, 1:4], in_=wk_d.ap()[:, 1:4])
            nc.gpsimd.dma_start(out=wq[:, 1:4], in_=wq_d.ap()[:, 1:4])
            ones_f = w1.tile([128, S], f32)
            nc.vector.memset(ones_f[:], 1.0)

            qT = persist.tile([128, MT, S], f16)
            kT = persist.tile([128, MT, S], f16)
            # V with an all-ones 65th channel: the PV matmul emits the
            # softmax rowsum at PSUM partition 64 for free (M=65);
            # whole-tile memset (strided column memsets misfire), V spills
            # overwrite channels 0-63, leaving channel 64 == 1.0
            vv = persist.tile([128, ST, HSH, 72], f16)
            nc.vector.memset(vv[:], 1.0)
            ones4 = persist.tile([97, HEAD_DIM], f16)
            nc.vector.tensor_copy(ones4[:], ones_f[0:97, 0:HEAD_DIM])

            # ---- phases 1+2: projections zippered into attention ----
            # K0/Q0/V run as a prologue; each attention (mt, c2) pass's
            # inner loop then carries later projection matmuls so PE fills
            # the gaps while ACT streams exps.
            attU = persist.tile([128, MT, S], f16)
            # rowsum reciprocals at partitions 32*(hh*2+c2)
            recq = persist.tile([97, MT, 512], f16)
            rs_sb = persist.tile([97, MT, 512], f32)
            recf = persist.tile([97, MT, 512], f32)
            recd = nc.dram_tensor("recd", [97, MT, 512], f16)

            with tc.tile_pool(name="p1ps", bufs=2, space="PSUM") as p1ps, \
                 tc.tile_pool(name="p1sb", bufs=3) as p1sb, \
                 tc.tile_pool(name="p2r", bufs=2) as p2r:

                def rope_apply(dst, b, c2, ps, pppool):
                    # row-tile 0 only: RoPE on the first 64 flat channels
                    # (rows 64-127 and the hg=1 core get identity via
                    # cos=1/sin=0 from the host).
                    sl = slice(c2 * 512, (c2 + 1) * 512)
                    sinp = p1sb.tile([128, 512], f16, tag="sinp", name="sinp")
                    nc.vector.scalar_tensor_tensor(
                        sinp[:], ps[:], b[:, 0:1],
                        sin2[:, sl], op0=OP.add, op1=OP.mult)
                    cosp = p1sb.tile([128, 512], f32, tag="cosp", name="cosp")
                    nc.vector.scalar_tensor_tensor(
                        cosp[:], ps[:], b[:, 0:1],
                        cos2[:, sl], op0=OP.add, op1=OP.mult)
                    pp = pppool.tile([128, 512], f32, tag="pp", name="pp")
                    nc.tensor.matmul(out=pp[:], lhsT=prt[:], rhs=sinp[:],
                                     start=True, stop=True)
                    nc.vector.tensor_tensor(
                        dst[:, 0, sl], cosp[:], pp[:], op=OP.add)

                def proj_v(st, pool):
                    # bv is folded in on the host: softmax weights sum to 1,
                    # so attn-out = attU/denom + bv exactly; bv@Wo joins bo.
                    ps = pool.tile([128, DSH], f32, tag="vps", name="ps")
                    for kt in range(KT):
                        nc.tensor.matmul(
                            out=ps[:],
                            lhsT=xT[:, kt, st * 128:(st + 1) * 128],
                            rhs=wv[:, kt, :],
                            start=(kt == 0), stop=(kt == KT - 1))
                    # spill on DVE: ACT's FIFO must stay clear so the first
                    # attention exps aren't queued behind these copies
                    nc.vector.tensor_copy(
                        vv[:, st, :, 0:HEAD_DIM],
                        ps[:].rearrange("p (h d) -> p h d", h=HSH))

                def proj_gen(mt):
                    # generator: one projection matmul per next(); bias-add
                    # epilogue rides with each group's final matmul.
                    for dst, w, b in ((kT, wk, bk), (qT, wq, bq)):
                        for c2 in range(2):
                            sl = slice(c2 * 512, (c2 + 1) * 512)
                            ps = p1ps.tile([128, 512], f32, tag="ps",
                                           name="ps")
                            for kt in range(KT):
                                nc.tensor.matmul(
                                    out=ps[:], lhsT=w[:, mt, kt, :],
                                    rhs=xT[:, kt, sl],
                                    start=(kt == 0), stop=(kt == KT - 1))
                                if kt == KT - 1:
                                    nc.vector.tensor_scalar(
                                        dst[:, mt, sl], ps[:], b[:, mt:mt + 1],
                                        None, op0=OP.add)
                                yield
                    while True:
                        yield

                # prologue: row-tile 0 projections + all of V, with a
                # deep PSUM pool (banks are free until the attention pools
                # open); all 32 K0/Q0 matmuls run dense, then the RoPE
                # chains consume the held PSUM tiles while V streams.
                with tc.tile_pool(name="p1pp", bufs=1,
                                  space="PSUM") as p1pp, \
                     tc.tile_pool(name="vps", bufs=5,
                                  space="PSUM") as vps:
                    kq_ps = []
                    for dst, w, b in ((kT, wk, bk), (qT, wq, bq)):
                        for c2 in range(2):
                            sl = slice(c2 * 512, (c2 + 1) * 512)
                            ps = vps.tile([128, 512], f32, tag="vps",
                                          name="ps")
                            for kt in range(KT):
                                nc.tensor.matmul(
                                    out=ps[:], lhsT=w[:, 0, kt, :],
                                    rhs=xT[:, kt, sl],
                                    start=(kt == 0), stop=(kt == KT - 1))
                            kq_ps.append((dst, b, c2, ps))
                    proj_v(0, vps)
                    for dst, b, c2, ps in kq_ps:
                        rope_apply(dst, b, c2, ps, p1pp)
                    for st in range(1, ST):
                        proj_v(st, vps)

                def normalize(mt, c2):
                    # DRAM bounce on the gpsimd DMA queue partition-
                    # broadcasts each head's 1/rowsum row; the scale runs on
                    # the otherwise-idle GpSimd engine to keep DVE clear.
                    qsl = slice(c2 * 512, (c2 + 1) * 512)
                    nc.gpsimd.dma_start(
                        out=recd.ap()[32 * c2:32 * c2 + 65:64, mt, :],
                        in_=recq[32 * c2:32 * c2 + 65:64, mt, :])
                    for hh in range(2):
                        ph = hh * 64
                        r = 32 * (hh * 2 + c2)
                        rb = p2r.tile([128, 512], f32, tag="rb", name="rb")
                        nc.gpsimd.dma_start(
                            out=rb[ph:ph + 64],
                            in_=recd.ap()[r:r + 1,
                                          mt, :].partition_broadcast(HEAD_DIM))
                        nc.gpsimd.tensor_tensor(
                            attU[ph:ph + 64, mt, qsl],
                            attU[ph:ph + 64, mt, qsl],
                            rb[ph:ph + 64], op=OP.mult)

                with tc.tile_pool(name="p2sc", bufs=2, space="PSUM") as p2sc, \
                     tc.tile_pool(name="p2at", bufs=1, space="PSUM") as p2at, \
                     tc.tile_pool(name="p2sb", bufs=4) as p2sb:

                    def emit_scores(mt, kt, c2):
                        # one [128,1024] tile per (q-half, key-chunk):
                        # h0 -> bank-half 0:512, h1 -> 512:1024 (adjacent
                        # row-group pair -> concurrent); both heads share
                        # the per-key exp bias so ONE exp covers the tile.
                        qsl = slice(c2 * 512, (c2 + 1) * 512)
                        sch = p2sc.tile([128, S], f32, tag="scc", name="scc")
                        for hh in range(2):
                            ph = hh * 64
                            nc.tensor.matmul(
                                out=sch[:, hh * 512:hh * 512 + 512],
                                lhsT=kT[ph:ph + 64, mt,
                                        kt * 128:(kt + 1) * 128],
                                rhs=qT[ph:ph + 64, mt, qsl],
                                start=True, stop=True,
                                tile_position=(ph, 0))
                        return sch

                    # q-halves run sequentially (16 key-chunk iterations per
                    # row-tile): DoubleRow PV must write PSUM partition 0, so
                    # only one q-half's pair of [65,512] accumulators fits
                    # alongside the double-buffered score tiles.
                    # nonzero through the last iterations so the PE has
                    # filler while it waits on the final exps of a pass
                    nproj_it = (3, 3, 3, 3, 2, 2, 2, 2, 2, 2, 2, 2, 1, 1, 1, 1)
                    passes = [(mt, c2) for mt in range(MT) for c2 in range(2)]
                    # wo is phase-3-only: its 1MB DMA is issued here so it
                    # does not compete with x/wq/wk/wv during the
                    # bandwidth-bound prologue window
                    nc.scalar.dma_start(out=wo[:], in_=wo_d.ap())
                    pending_recip = []
                    sch_next = emit_scores(0, 0, 0)
                    for pi, (mt, c2) in enumerate(passes):
                        if c2 == 0:
                            gen = proj_gen(mt + 1) if mt + 1 < MT else iter(
                                lambda: None, 0)  # infinite no-op iterator
                        if True:
                            qsl = slice(c2 * 512, (c2 + 1) * 512)
                            at = {hh: p2at.tile([65, 512], f32,
                                                name=f"at{hh}", tag=f"at{hh}")
                                  for hh in range(2)}
                            sch = sch_next
                            pt2 = None
                            for kt in range(ST):
                                pt2 = p2sb.tile([128, S], f16,
                                                tag="ptc", name="ptc")
                                nc.scalar.activation(
                                    pt2[:], sch[:], AF.Exp,
                                    bias=maskb[:, kt:kt + 1], scale=0.125)
                                first, last = (kt == 0), (kt == ST - 1)
                                if kt == 1 and pending_recip:
                                    # previous pass's reciprocal chain runs
                                    # here so it never gates the DVE at the
                                    # pass boundary; its consumer (the kt==2
                                    # DMA-bounce normalize) is still ahead
                                    pr = pending_recip.pop(0)
                                    nc.vector.reciprocal_approx_fast(
                                        recf[:, pr, :], rs_sb[:, pr, :])
                                    nc.vector.tensor_copy(recq[:, pr, :],
                                                          recf[:, pr, :])
                                if not last:
                                    sch = emit_scores(mt, kt + 1, c2)
                                elif pi + 1 < len(passes):
                                    # cross-pass prefetch: the next pass's
                                    # first score tile, so ACT rolls into
                                    # its exps with no pipeline drain
                                    sch_next = emit_scores(
                                        passes[pi + 1][0], 0,
                                        passes[pi + 1][1])
                                for _ in range(nproj_it[c2 * ST + kt]):
                                    next(gen)
                                for hh in range(2):  # PV+rowsum (M=65)
                                    nc.tensor.matmul(
                                        out=at[hh][:],
                                        lhsT=vv[:, kt, mt * 2 + hh, 0:65],
                                        rhs=pt2[:, hh * 512:hh * 512 + 512],
                                        start=first, stop=last)
                                if kt == 2 and c2 == 0 and mt > 0:
                                    normalize(mt - 1, 1)
                                if kt == 2 and c2 == 1:
                                    normalize(mt, 0)
                            # epilogue for this (mt, c2): stage the PSUM
                            # rowsum rows to SBUF (the custom-DVE recip
                            # mangles offset single-partition PSUM reads),
                            # one full-height reciprocal (junk partitions
                            # between the used rows are never read), then
                            # spill the attn columns
                            # rowsum staging + attn spills first -- they are
                            # the last readers of the PSUM accumulators, so
                            # the next pass's PV chain unblocks as soon as
                            # they retire; the reciprocal chain (only needed
                            # at the next pass's kt==2 bounce) runs after.
                            last_half = (mt == MT - 1 and c2 == 1)
                            for hh in range(2):
                                r = 32 * (hh * 2 + c2)
                                ph = hh * 64
                                if hh == 1 and not last_half and pi < 5:
                                    nc.scalar.activation(
                                        rs_sb[r:r + 1, mt, :],
                                        at[hh][64:65, :], AF.Copy)
                                else:
                                    nc.vector.tensor_copy(
                                        rs_sb[r:r + 1, mt, :],
                                        at[hh][64:65, :])
                                # one spill rides ACT when the следующий pass
                                # still has exp slack (mt0-2), halving the
                                # DVE work that gates PSUM reuse at the
                                # pass boundary
                                on_act = ((last_half and hh == 0) or
                                          (not last_half and hh == 1 and
                                           pi < 5))
                                if on_act:
                                    nc.scalar.activation(
                                        attU[ph:ph + 64, mt, qsl],
                                        at[hh][0:64, :], AF.Copy)
                                else:
                                    nc.vector.tensor_copy(
                                        attU[ph:ph + 64, mt, qsl],
                                        at[hh][0:64, :])
                            if last_half:
                                nc.vector.reciprocal_approx_fast(
                                    recf[:, mt, :], rs_sb[:, mt, :])
                                nc.vector.tensor_copy(recq[:, mt, :],
                                                      recf[:, mt, :])
                            else:
                                pending_recip.append(mt)

        # ---- phase 3: output projection -------------------------------
        # bo (and bv@Wo) are added on the host, so each (qt, c2) tile is a
        # bare 4-matmul accumulation chain; f16 output halves the out DMA.
        with tc.tile_pool(name="p3ps", bufs=8, space="PSUM") as p3ps, \
             tc.tile_pool(name="p3sb", bufs=3) as p3sb:
            # qt 0-3 read only attU columns 0-511 (normalized during the
            # c2=1 pass via the DMA bounce), so their chains are emitted
            # FIRST; the c2=1-half broadcast normalize runs on DVE under
            # them, unblocking qt 4-7.
            def final_norm():
                for hh in range(2):
                    ph = hh * 64
                    r = 32 * (hh * 2 + 1)
                    rbps = p3ps.tile([HEAD_DIM, 512], f32, tag="ps3",
                                     name="rbps")
                    nc.tensor.matmul(
                        out=rbps[:], lhsT=ones4[r:r + 1, :],
                        rhs=recq[r:r + 1, MT - 1, :],
                        start=True, stop=True, tile_position=(r, 0))
                    nc.vector.tensor_tensor(
                        attU[ph:ph + 64, MT - 1, 512:1024],
                        attU[ph:ph + 64, MT - 1, 512:1024],
                        rbps[:], op=OP.mult)
            dmaq = [nc.sync, nc.scalar, nc.gpsimd]
            for qt in list(range(4)) + ["norm"] + list(range(4, ST)):
                if qt == "norm":
                    final_norm()
                    continue
                ob = p3sb.tile([128, DIM], f16, tag="ob")
                for c2 in range(DIM // 512):
                    nsl = slice(c2 * 512, (c2 + 1) * 512)
                    ps = p3ps.tile([128, 512], f32, tag="ps3", name="ps3")
                    for mt in range(MT):
                        nc.tensor.matmul(
                            out=ps[:],
                            lhsT=attU[:, mt, qt * 128:(qt + 1) * 128],
                            rhs=wo[:, mt, nsl],
                            start=(mt == 0), stop=(mt == MT - 1))
                    # masked-row zeroing fused into the PSUM->SBUF move, on
                    # ACT (idle in this phase) to keep DVE off the path
                    nc.scalar.activation(
                        ob[:, nsl], ps[:], AF.Copy,
                        scale=mask01[:, qt:qt + 1])
                    dmaq[(qt * 2 + c2) % 3].dma_start(
                        out=out_d.ap()[qt * 128:(qt + 1) * 128, nsl],
                        in_=ob[:, nsl])

    nc.compile()
    return nc


def _get_nc():
    if "nc" not in _CACHE:
        _CACHE["nc"] = _build()
    return _CACHE["nc"]


def _prep_inputs(x, mask, freqs, Wq, bq, Wk, bk, Wv, bv, Wo, bo):
    f = np.asarray(freqs, np.float32)[0]              # [S, HEAD_DIM]
    # reference rotates only the first rot_dim=64 channels of the FLAT
    # inner dim -> rows 0-63 of row-tile 0 on the hg=0 core; everything
    # else is identity (cos=1, sin=0).
    cos2 = np.ones((128, S), np.float32)
    sin2 = np.zeros((128, S), np.float32)
    cos2[0:HEAD_DIM] = np.cos(f.T)
    sin2[0:HEAD_DIM] = np.sin(f.T)
    ident = np.ones((128, S), np.float32)
    identz = np.zeros((128, S), np.float32)

    prt = np.zeros((128, 128), np.float16)            # P_rot^T
    i = np.arange(0, 128, 2)
    prt[i + 1, i] = -1.0                              # P_rot[2i, 2i+1] = -1
    prt[i, i + 1] = 1.0                               # P_rot[2i+1, 2i] = +1

    def lhsT_w(w):                                    # [DIM, DSH] -> lhsT tiles
        # mt-major so the row-tile-0 slice is one contiguous DMA chunk
        return np.ascontiguousarray(
            w.reshape(KT, 128, MT, 128).transpose(1, 2, 0, 3)).astype(np.float16)

    def col(b):                                       # [DSH] -> [128, MT]
        return np.ascontiguousarray(b.reshape(MT, 128).T.astype(np.float32))

    in_maps = []
    for b in range(B):
        xT = np.ascontiguousarray(
            np.asarray(x[b], np.float32).T.reshape(KT, 128, S)
            .transpose(1, 0, 2)).astype(np.float16)
        m = np.asarray(mask[b])
        maskb = np.ascontiguousarray(
            np.where(m, 0.0, MASK_NEG).astype(np.float32).reshape(ST, 128).T)
        mask01 = np.ascontiguousarray(
            m.astype(np.float32).reshape(ST, 128).T)
        for hg in range(HG):
            dsl = slice(hg * DSH, (hg + 1) * DSH)
            in_maps.append({
                "xT": xT,
                "wq": lhsT_w(np.asarray(Wq, np.float32)[:, dsl]),
                "wk": lhsT_w(np.asarray(Wk, np.float32)[:, dsl]),
                "wv": np.ascontiguousarray(
                    np.asarray(Wv, np.float32)[:, dsl]
                    .reshape(KT, 128, DSH).transpose(1, 0, 2)).astype(np.float16),
                "wo": np.ascontiguousarray(
                    np.asarray(Wo, np.float32)[dsl, :]
                    .reshape(MT, 128, DIM).transpose(1, 0, 2)).astype(np.float16),
                "bq": col(np.asarray(bq, np.float32)[dsl]),
                "bk": col(np.asarray(bk, np.float32)[dsl]),
                "cos2": cos2 if hg == 0 else ident,
                "sin2": sin2 if hg == 0 else identz,
                "prt": prt,
                "maskb": maskb, "mask01": mask01,
            })
    return in_maps


def run(trace=False, **inputs):
    from concourse import bass_utils
    if trace:
        _install_ntff_hook()
    nc = _get_nc()
    in_maps = _prep_inputs(**inputs)
    res = bass_utils.run_bass_kernel_spmd(
        nc, in_maps, core_ids=list(range(NCORES)), trace=trace)
    # host epilogue: sum the two head-group partials and add the biases the
    # device never sees (bo, plus bv@Wo -- softmax weights sum to 1, so the
    # v-bias shifts every attn-out row by exactly bv); masked rows are 0.
    bias = (np.asarray(inputs["bo"], np.float32)
            + np.asarray(inputs["bv"], np.float32)
            @ np.asarray(inputs["Wo"], np.float32))
    mask = np.asarray(inputs["mask"])
    out = np.empty((B, S, DIM), np.float32)
    for b in range(B):
        s = (res.results[2 * b]["out"].astype(np.float32)
             + res.results[2 * b + 1]["out"].astype(np.float32) + bias)
        out[b] = np.where(mask[b][:, None], s, 0.0)
    return out, res


def kernel(**inputs):
    out, _ = run(trace=False, **inputs)
    return out


def _install_ntff_hook():
    """Register the axon NTFF profiling hook missing from the antenv stub."""
    import sys, types
    try:
        import antenv.axon_hooks  # noqa: F401
        return
    except ImportError:
        pass
    from trn_agent_boot.trn_boot import _ntff_profile_via_ctypes
    hook = _ntff_profile_via_ctypes('/opt/axon/libaxon_pjrt.so')
    mod = types.ModuleType('antenv.axon_hooks')
    mod.get_axon_ntff_profile_hook = lambda: hook
    mod.set_axon_ntff_profile_hook = lambda h: None
    sys.modules['antenv.axon_hooks'] = mod

